# revision 1
# baseline (speedup 1.0000x reference)
"""Trainium2 Bass kernel for nn_BaselineMemory (sparse attention memory read + MLP).

Data-parallel over batch: each of 8 NeuronCores handles 256 of 2048 rows.
Pipeline per core:
  x-norm (ACT) -> dist matmul z = x_hat @ y_hat^T - 1 (fp32r, PE)
  -> sparsemax via log-secant threshold iteration (ACT relu+bias+accum on the
     head of m, DVE chunked max+sum on the tail: S = sum max(z,tau) - n*tau)
  -> w^T transpose (PE) -> memory read mv^T (bf16, PE)
  -> MLP1 (bf16, transposed layout; b1 fused as per-partition ACT bias + ReLU)
  -> MLP2 (bf16; b2 via rank-1 fp32r matmul) -> fp32 out.
"""
import sys

if "/opt/trn_rl_repo" not in sys.path:
    sys.path.insert(0, "/opt/trn_rl_repo")

import numpy as np
import ml_dtypes

import concourse.bass as bass  # noqa: F401
import concourse.tile as tile
from concourse import bacc, mybir
from concourse.bass_utils import run_bass_kernel_spmd
from concourse.masks import make_identity

P = 128
B_CORE = 256          # batch rows per core
NBT = B_CORE // P     # 2 b-tiles
D = 1024
DC = D // P           # 8 d-chunks
M = 8192
MC512 = M // 512      # 16 m-chunks for dist
MC128 = M // P        # 64 m-chunks for read
H = 2048
HC = H // P           # 16 h-chunks
OUT = 1000
NH = 2                # out halves of 500
N_SECANT = 7          # secant iterations after the init pass
MA = 6144             # ACT handles m [0, MA); DVE chunks handle [MA, M)
NDV = (M - MA) // 512  # 7 DVE chunks of 512

F32 = mybir.dt.float32
F32R = mybir.dt.float32r
BF16 = mybir.dt.bfloat16
AF = mybir.ActivationFunctionType
ALU = mybir.AluOpType
AX = mybir.AxisListType
bf16 = ml_dtypes.bfloat16

_EPS = 1e-6


def build():
    nc = bacc.Bacc("TRN2", target_bir_lowering=False, debug=False)

    x_s = nc.dram_tensor("x_s", [NBT, P, D], F32, kind="ExternalInput")
    memT = nc.dram_tensor("memT", [DC, P, M], F32R, kind="ExternalInput")
    mem_bf = nc.dram_tensor("mem_bf", [MC128, P, D], BF16, kind="ExternalInput")
    # host-prepped partition-major layouts (contiguous per-partition runs)
    w1_bf = nc.dram_tensor("w1_bf", [P, DC, HC, P], BF16, kind="ExternalInput")
    w2_bf = nc.dram_tensor("w2_bf", [P, HC, OUT], BF16, kind="ExternalInput")
    b1_t = nc.dram_tensor("b1_t", [P, HC], F32, kind="ExternalInput")
    b2_r = nc.dram_tensor("b2_r", [1, OUT], F32R, kind="ExternalInput")
    out_d = nc.dram_tensor("out", [NBT, P, OUT], F32, kind="ExternalOutput")

    with tile.TileContext(nc) as tc:
        small = tc.alloc_tile_pool(name="small", bufs=1)
        pers = tc.alloc_tile_pool(name="pers", bufs=1)

        ident = small.tile([P, P], F32, tag="ident")
        make_identity(nc, ident[:])
        eps_t = small.tile([P, 1], F32, tag="eps")
        nc.vector.memset(eps_t[:], _EPS)
        b1t = small.tile([P, HC], F32, tag="b1")
        nc.sync.dma_start(b1t[:], b1_t[:])
        b2t = small.tile([1, OUT], F32R, tag="b2")
        nc.sync.dma_start(b2t[:], b2_r[:])
        ones1f = small.tile([1, P], F32, tag="ones1f")
        nc.vector.memset(ones1f[:], 1.0)
        ones1 = small.tile([1, P], F32R, tag="ones1")
        nc.vector.tensor_copy(ones1[:], ones1f[:])

        # ---- x load + normalize + transpose -> xnT [P, dc, 256] fp32r ----
        xnT = pers.tile([P, DC, B_CORE], F32R, tag="xnT")
        xpool = tc.alloc_tile_pool(name="xpool", bufs=1)
        ps_x = tc.alloc_tile_pool(name="ps_x", bufs=2, space="PSUM")
        xn = []
        inv_x = []
        for bt in range(NBT):
            xt = xpool.tile([P, D], F32, tag=f"x{bt}")
            nc.sync.dma_start(xt[:], x_s[bt])
            ss = small.tile([P, 1], F32, tag=f"ss{bt}")
            sq = xpool.tile([P, D], F32, tag="sqscratch")
            nc.scalar.activation(sq[:], xt[:], AF.Square, accum_out=ss[:])
            nrm = small.tile([P, 1], F32, tag=f"nrm{bt}")
            nc.scalar.activation(nrm[:], ss[:], AF.Sqrt, bias=eps_t[:, 0:1])
            inv = small.tile([P, 1], F32, tag=f"inv{bt}")
            nc.vector.reciprocal(inv[:], nrm[:])
            inv_x.append(inv)
            xn.append(xt)
        # transpose RAW x (row scaling folded into the z evacuation; sparsemax
        # is shift-invariant so the -1 is dropped entirely: z = cos)
        for dc in range(DC):
            pt = ps_x.tile([P, B_CORE], F32, tag="xtr")
            for bt in range(NBT):
                nc.tensor.transpose(
                    pt[:, bt * P:(bt + 1) * P],
                    xn[bt][:, dc * P:(dc + 1) * P], ident[:])
            nc.vector.tensor_copy(xnT[:, dc], pt[:])
        ps_x.release()
        xpool.release()

        # Slot-sharing pools: wpool tags w0/w1 (32KB slots), zpool tags z0/z1.
        wpool = tc.alloc_tile_pool(name="wpool", bufs=1)
        w = [wpool.tile([P, M], F32, tag=f"w{bt}", name=f"w{bt}") for bt in range(NBT)]
        zpool = tc.alloc_tile_pool(name="zpool", bufs=1)
        z = [zpool.tile([P, M], F32, tag=f"z{bt}", name=f"z{bt}") for bt in range(NBT)]
        mstream = tc.alloc_tile_pool(name="mstream", bufs=2)

        # ---- dist matmul: z[bt] [P, M] fp32 (= cos - 1) + chunk maxes + sums ----
        mx = [small.tile([P, MC512], F32, tag=f"mx{bt}", name=f"mx{bt}")
              for bt in range(NBT)]
        zsum = [small.tile([P, MC512], F32, tag=f"zs{bt}", name=f"zs{bt}")
                for bt in range(NBT)]
        ps_dist = tc.alloc_tile_pool(name="ps_dist", bufs=6, space="PSUM")
        for mc in range(MC512):
            mtile = mstream.tile([P, DC, 512], F32R, tag="memT")
            for dq in range(4):
                nc.sync.dma_start(
                    mtile[:, dq * 2:(dq + 1) * 2],
                    memT[dq * 2:(dq + 1) * 2, :, mc * 512:(mc + 1) * 512]
                    .rearrange("d p m -> p d m"))
            for bt in range(NBT):
                zp = ps_dist.tile([P, 512], F32, tag="zp")
                for dc in range(DC):
                    nc.tensor.matmul(
                        zp[:], xnT[:, dc, bt * P:(bt + 1) * P], mtile[:, dc],
                        start=(dc == 0), stop=(dc == DC - 1))
                nc.vector.tensor_scalar(
                    out=z[bt][:, mc * 512:(mc + 1) * 512], in0=zp[:],
                    scalar1=inv_x[bt][:, 0:1], scalar2=None,
                    op0=ALU.mult, op1=ALU.add,
                    accum_out=zsum[bt][:, mc:mc + 1])
                nc.vector.reduce_max(
                    mx[bt][:, mc:mc + 1], zp[:], axis=AX.X)
        ps_dist.release()

        # ---- sparsemax via log-secant; S(tau) = ACT head + DVE tail chunks ----
        ps_warm = tc.alloc_tile_pool(name="ps_warm", bufs=2, space="PSUM")
        CAP_OFF = 1e-4

        tail_scr = [small.tile([P, M - MA], F32, tag=f"tailscr{b}", name=f"ts{b}")
                    for b in range(NBT)]

        def s_pass(bt, stt):
            tau_c, ntau = stt["tau_c"], stt["ntau"]
            s_act, gacc, s_v = stt["s_act"], stt["gacc"], stt["s_v"]
            nc.scalar.activation(
                w[bt][:, 0:MA], z[bt][:, 0:MA], AF.Relu,
                bias=ntau[:, 0:1], accum_out=s_act[:])
            # relu in two DVE ops: max into scratch, then (x - tau) with
            # fused add-reduce accum (sums small positives -> no cancellation)
            nc.vector.tensor_scalar(
                out=tail_scr[bt][:], in0=z[bt][:, MA:M],
                scalar1=tau_c[:, 0:1], scalar2=None, op0=ALU.max)
            nc.vector.tensor_scalar(
                out=w[bt][:, MA:M], in0=tail_scr[bt][:],
                scalar1=tau_c[:, 0:1], scalar2=None,
                op0=ALU.subtract, op1=ALU.add, accum_out=gacc[:, 0:1])
            nc.vector.tensor_add(s_v[:], gacc[:, 0:1], s_act[:])
            # PE warmers: keep HAM at 8/8 through the sparsemax window
            wp = ps_warm.tile([P, 512], F32, tag="warm")
            nc.tensor.matmul(wp[:], ident[:], w[bt][:, 0:512],
                             start=True, stop=True)
            nc.tensor.matmul(wp[:], ident[:], w[bt][:, 512:1024],
                             start=True, stop=True)

        st = {}
        for bt in range(NBT):
            stt = {}
            for nm in ["rm", "cap", "tau_p", "tau_c", "l_p", "l_c", "ntau",
                       "s_v", "s_act", "gs", "corr", "dl", "rdl", "dt",
                       "step", "neg"]:
                stt[nm] = small.tile([P, 1], F32, tag=f"{nm}{bt}", name=f"{nm}{bt}")
            stt["gacc"] = small.tile([P, NDV], F32, tag=f"gacc{bt}", name=f"gacc{bt}")
            st[bt] = stt
            rm, cap, tau_p, tau_c = stt["rm"], stt["cap"], stt["tau_p"], stt["tau_c"]
            l_p, ntau, s_v = stt["l_p"], stt["ntau"], stt["s_v"]
            nc.vector.reduce_max(rm[:], mx[bt][:], axis=AX.X)
            nc.vector.tensor_mul(rm[:], rm[:], inv_x[bt][:])  # rowmax of z=cos
            nc.vector.tensor_scalar_add(cap[:], rm[:], -CAP_OFF)
            nc.vector.tensor_scalar_add(tau_p[:], rm[:], -1.0)
            # analytic S0 = sum(z) - M*tau_p (tau_p = rowmax-1; <= true S, safe)
            zs = stt["gs"]
            nc.vector.reduce_sum(zs[:], zsum[bt][:], axis=AX.X)
            nc.vector.tensor_scalar_mul(s_v[:], tau_p[:], -float(M))
            nc.vector.tensor_add(s_v[:], s_v[:], zs[:])
            nc.vector.tensor_scalar_max(s_v[:], s_v[:], 1.0)  # guard ln<=0
            nc.scalar.activation(l_p[:], s_v[:], AF.Ln)
            # tau_c = tau_p + (S0-1)/M, capped
            nc.vector.tensor_scalar(
                out=tau_c[:], in0=s_v[:], scalar1=-1.0, scalar2=1.0 / M,
                op0=ALU.add, op1=ALU.mult)
            nc.vector.tensor_add(tau_c[:], tau_c[:], tau_p[:])
            nc.vector.tensor_tensor(tau_c[:], tau_c[:], cap[:], ALU.min)
            nc.vector.tensor_scalar_mul(ntau[:], tau_c[:], -1.0)
            s_pass(bt, stt)
            nc.scalar.activation(stt["l_c"][:], s_v[:], AF.Ln)

        for it in range(N_SECANT):
            for bt in range(NBT):
                stt = st[bt]
                cap, tau_p, tau_c = stt["cap"], stt["tau_p"], stt["tau_c"]
                l_p, l_c, ntau = stt["l_p"], stt["l_c"], stt["ntau"]
                dl, rdl, dt = stt["dl"], stt["rdl"], stt["dt"]
                step = stt["step"]
                nc.vector.tensor_sub(dl[:], l_p[:], l_c[:])
                nc.vector.tensor_scalar(
                    out=dl[:], in0=dl[:], scalar1=-1.0, scalar2=dl[:, 0:1],
                    op0=ALU.mult, op1=ALU.max)   # |dl|
                nc.vector.tensor_scalar_max(dl[:], dl[:], 1e-12)
                nc.vector.reciprocal(rdl[:], dl[:])
                nc.vector.tensor_sub(dt[:], tau_c[:], tau_p[:])
                nc.vector.tensor_scalar(
                    out=dt[:], in0=dt[:], scalar1=-1.0, scalar2=dt[:, 0:1],
                    op0=ALU.mult, op1=ALU.max)   # |dt|
                nc.vector.tensor_scalar(
                    out=step[:], in0=dt[:], scalar1=rdl[:, 0:1], scalar2=2.0,
                    op0=ALU.mult, op1=ALU.min)   # q = min(|dt|/|dl|, 2)
                nc.vector.tensor_copy(tau_p[:], tau_c[:])
                nc.vector.tensor_mul(step[:], step[:], l_c[:])
                nc.vector.tensor_scalar(
                    out=tau_c[:], in0=step[:], scalar1=tau_c[:, 0:1],
                    scalar2=cap[:, 0:1], op0=ALU.add, op1=ALU.min)
                nc.vector.tensor_copy(l_p[:], l_c[:])
                nc.vector.tensor_scalar_mul(ntau[:], tau_c[:], -1.0)
                s_pass(bt, stt)  # last iteration leaves w (head exact relu)
            if it != N_SECANT - 1:
                # Lns emitted after BOTH passes: avoids ACT FIFO head-of-line
                # blocking (Ln waits on the DVE tail sum; a pass queued behind
                # it would stall)
                for bt in range(NBT):
                    nc.scalar.activation(st[bt]["l_c"][:], st[bt]["s_v"][:], AF.Ln)
        ps_warm.release()

        # prefetch first mem slab during the sparsemax/transpose window
        mtile0 = mstream.tile([P, 4, D], BF16, tag="memT", name="membf0")
        for dq in range(2):
            nc.sync.dma_start(
                mtile0[:, dq * 2:(dq + 1) * 2],
                mem_bf[dq * 2:(dq + 1) * 2].rearrange("c p d -> p c d"))

        # ---- wT transposes interleaved with read matmuls (per-mc pipeline) ----
        wTt = zpool.tile([P, MC128, B_CORE], BF16, tag="z0", name="wTt")
        w1t = zpool.tile([P, DC, HC, P], BF16, tag="z1", name="w1t")
        for dq in range(4):
            nc.sync.dma_start(w1t[:, dq * 2:(dq + 1) * 2],
                              w1_bf[:, dq * 2:(dq + 1) * 2])
        ps_trw = tc.alloc_tile_pool(name="ps_trw", bufs=4, space="PSUM")
        ps_mv = tc.alloc_tile_pool(name="ps_mv", bufs=1, space="PSUM")
        mv_ps = [[ps_mv.tile([P, 512], F32, tag=f"mv{bt}_{dh}", name=f"mv{bt}_{dh}")
                  for dh in range(2)] for bt in range(NBT)]
        for mc4 in range(MC128 // 4):
            if mc4 == 0:
                mtile = mtile0
            else:
                mtile = mstream.tile([P, 4, D], BF16, tag="memT", name="membf")
                for dq in range(2):
                    nc.sync.dma_start(
                        mtile[:, dq * 2:(dq + 1) * 2],
                        mem_bf[mc4 * 4 + dq * 2:mc4 * 4 + (dq + 1) * 2]
                        .rearrange("c p d -> p c d"))
            for c in range(4):
                mc = mc4 * 4 + c
                tp = ps_trw.tile([P, B_CORE], F32, tag="wtr")
                for bt in range(NBT):
                    nc.tensor.transpose(
                        tp[:, bt * P:(bt + 1) * P],
                        w[bt][:, mc * P:(mc + 1) * P], ident[:])
                if mc % 2 == 0:
                    nc.vector.tensor_copy(wTt[:, mc], tp[:])
                else:
                    nc.scalar.copy(wTt[:, mc], tp[:])
                for bt in range(NBT):
                    for dh in range(2):
                        nc.tensor.matmul(
                            mv_ps[bt][dh][:],
                            wTt[:, mc, bt * P:(bt + 1) * P],
                            mtile[:, c, dh * 512:(dh + 1) * 512],
                            start=(mc == 0), stop=(mc == MC128 - 1))

        # evacuate mv to fp32 SBUF, transpose to mvT bf16 [P, dc, 256]
        mv_sb = [small.tile([P, D], F32, tag=f"mvsb{bt}", name=f"mvsb{bt}")
                 for bt in range(NBT)]
        for bt in range(NBT):
            for dh in range(2):
                nc.scalar.copy(mv_sb[bt][:, dh * 512:(dh + 1) * 512],
                               mv_ps[bt][dh][:])
        ps_mv.release()
        ps_trw.release()
        mvT = wpool.tile([P, DC, B_CORE], BF16, tag="w0", name="mvT")
        ps_mvt = tc.alloc_tile_pool(name="ps_mvt", bufs=4, space="PSUM")
        for dc in range(DC):
            tp = ps_mvt.tile([P, B_CORE], F32, tag="mvtr")
            for bt in range(NBT):
                nc.tensor.transpose(
                    tp[:, bt * P:(bt + 1) * P],
                    mv_sb[bt][:, dc * P:(dc + 1) * P], ident[:])
            nc.vector.tensor_copy(mvT[:, dc], tp[:])
        ps_mvt.release()

        # ---- MLP1: hT[hc] = relu(sum_dc W1-block^T @ mvT[dc] + b1[hc]) ----
        hT = wpool.tile([P, HC, B_CORE], BF16, tag="w1", name="hT")
        ps_h = tc.alloc_tile_pool(name="ps_h", bufs=4, space="PSUM")
        for hc in range(HC):
            hp = ps_h.tile([P, B_CORE], F32, tag="hp")
            for dc in range(DC):
                nc.tensor.matmul(
                    hp[:], w1t[:, dc, hc], mvT[:, dc],
                    start=(dc == 0), stop=(dc == DC - 1))
            nc.scalar.activation(
                hT[:, hc], hp[:], AF.Relu, bias=b1t[:, hc:hc + 1])
        ps_h.release()

        # ---- MLP2: out[bt] = hT-blocks^T @ W2 + b2 (nh outer, W2 slab DMA) ----
        ps_o = tc.alloc_tile_pool(name="ps_o", bufs=4, space="PSUM")
        osb = [small.tile([P, OUT], F32, tag=f"osb{bt}", name=f"osb{bt}")
               for bt in range(NBT)]
        NW = OUT // NH
        w2slabs = []
        for nh in range(NH):
            w2slab = mstream.tile([P, HC, NW], BF16, tag="memT",
                                  name=f"w2slab{nh}")
            for dq in range(2):
                nc.sync.dma_start(
                    w2slab[:, dq * 8:(dq + 1) * 8],
                    w2_bf[:, dq * 8:(dq + 1) * 8, nh * NW:(nh + 1) * NW])
            w2slabs.append(w2slab)
        for nh in range(NH):
            w2slab = w2slabs[nh]
            ops = [ps_o.tile([P, NW], F32, tag=f"op{bt}", name=f"op{bt}")
                   for bt in range(NBT)]
            for kc in range(HC):
                for bt in range(NBT):
                    nc.tensor.matmul(
                        ops[bt][:], hT[:, kc, bt * P:(bt + 1) * P],
                        w2slab[:, kc], start=(kc == 0), stop=False)
            for bt in range(NBT):
                nc.tensor.matmul(
                    ops[bt][:], ones1[:], b2t[:, nh * NW:(nh + 1) * NW],
                    start=False, stop=True)
                nc.scalar.copy(osb[bt][:, nh * NW:(nh + 1) * NW], ops[bt][:])
                nc.sync.dma_start(
                    out_d[bt, :, nh * NW:(nh + 1) * NW],
                    osb[bt][:, nh * NW:(nh + 1) * NW])
        ps_o.release()

        mstream.release()
        zpool.release()
        wpool.release()
        pers.release()
        small.release()

    nc.compile()
    return nc


_CACHED = None


def _prep(inputs):
    x = np.ascontiguousarray(inputs["encoder_output"], dtype=np.float32)
    mem = np.ascontiguousarray(inputs["memory_set"], dtype=np.float32)
    W1 = np.ascontiguousarray(inputs["W1"], dtype=np.float32)
    b1 = np.ascontiguousarray(inputs["b1"], dtype=np.float32)
    W2 = np.ascontiguousarray(inputs["W2"], dtype=np.float32)
    b2 = np.ascontiguousarray(inputs["b2"], dtype=np.float32)

    inv_ny = 1.0 / np.sqrt((mem * mem).sum(1) + _EPS)
    memT_hat = np.ascontiguousarray(
        (mem.T * inv_ny[None, :]).astype(np.float32).reshape(DC, P, M))
    mem_bfv = np.ascontiguousarray(mem.astype(bf16).reshape(MC128, P, D))
    # partition-major blocks: w1[p, dc, hc, c] = W1[dc*128+p, hc*128+c]
    w1_blk = np.ascontiguousarray(
        W1.astype(bf16).reshape(DC, P, HC, P).transpose(1, 0, 2, 3))
    # w2[p, kc, o] = W2[kc*128+p, o]
    w2_blk = np.ascontiguousarray(
        W2.astype(bf16).reshape(HC, P, OUT).transpose(1, 0, 2))
    b1_tiles = np.ascontiguousarray(b1.reshape(HC, P).T.astype(np.float32))
    b2_row = np.ascontiguousarray(b2.reshape(1, OUT).astype(np.float32))

    shared = {
        "memT": memT_hat, "mem_bf": mem_bfv, "w1_bf": w1_blk,
        "w2_bf": w2_blk, "b1_t": b1_tiles, "b2_r": b2_row,
    }
    in_maps = []
    for c in range(8):
        xs = np.ascontiguousarray(
            x[c * B_CORE:(c + 1) * B_CORE].reshape(NBT, P, D))
        in_maps.append({"x_s": xs, **shared})
    return in_maps


def kernel(**inputs) -> np.ndarray:
    global _CACHED
    if _CACHED is None:
        _CACHED = build()
    nc = _CACHED
    in_maps = _prep(inputs)
    res = run_bass_kernel_spmd(nc, in_maps, core_ids=list(range(8)))
    return np.concatenate(
        [r["out"].reshape(B_CORE, OUT) for r in res.results], axis=0)



# revision 7
# speedup vs baseline: 1.0659x; 1.0659x over previous
"""Trainium2 Bass kernel for nn_BaselineMemory (sparse attention memory read + MLP).

Data-parallel over batch: each of 8 NeuronCores handles 256 of 2048 rows.
fp16 end-to-end (fp32 PSUM/accum). Host pre-normalizes x and mem^T.
Pipeline per core:
  dist matmul z = x_hat @ y_hat^T (fp16, PE) -> z fp16 + block sums/maxes
  -> sparsemax tau via 3 Newton rounds from a Gaussian-moment init
     (S(tau) = ACT relu head + DVE tail; support count k on DVE is_ge;
      tau += (S-1)/k) -> w materialization pass
  -> w^T transposes (PE, fp16) -> memory read mv^T (fp16, PE)
  -> MLP1 (W1 natural layout, relu+b1 fused into post-transpose evac)
  -> MLP2 (+b2 via rank-1 matmul) -> fp32 out.
DMA: memT stream + memR ring + outputs on the Sync queue; W1/W2 on the
GpSimd queue so they transfer during the sparsemax window.
"""
import sys

if "/opt/trn_rl_repo" not in sys.path:
    sys.path.insert(0, "/opt/trn_rl_repo")

import numpy as np

import concourse.bass as bass  # noqa: F401
import concourse.tile as tile
from concourse import bacc, mybir
from concourse.bass_utils import run_bass_kernel_spmd
from concourse.masks import make_identity

P = 128
B_CORE = 256          # batch rows per core
NBT = B_CORE // P     # 2 b-tiles
D = 1024
DC = D // P           # 8 d-chunks
M = 8192
MB = M // 512         # 16 dist m-blocks per bt
MC = M // P           # 64 m-chunks for read
H = 2048
HC = H // P           # 16 h-chunks
HB = H // 512         # 4 mlp1 col-blocks
OUT = 1000
NOH = 2               # out halves of 500
NW = OUT // NOH

N_ROUNDS = 3          # Newton iterations on tau
MA = 4608             # S-pass head handled by ACT; tail [MA, M) on DVE
T0_SIG = 2.25 / 32.0  # init: tau0 = mean + 2.25*sigma, sigma = 1/sqrt(d)
CAP_OFF = 1e-4
RING = 24             # memR ring slabs resident
W2BUF = 8

F32 = mybir.dt.float32
F16 = mybir.dt.float16
AF = mybir.ActivationFunctionType
ALU = mybir.AluOpType
AX = mybir.AxisListType

_EPS = 1e-6


def build():
    nc = bacc.Bacc("TRN2", target_bir_lowering=False, debug=False)

    xhT_d = nc.dram_tensor("xhT", [DC, P, B_CORE], F16, kind="ExternalInput")
    memT = nc.dram_tensor("memT", [DC, P, M], F16, kind="ExternalInput")
    memR = nc.dram_tensor("memR", [MC, P, D], F16, kind="ExternalInput")
    w1c = nc.dram_tensor("w1c", [DC, P, H], F16, kind="ExternalInput")
    w2c = nc.dram_tensor("w2c", [HC, P, OUT], F16, kind="ExternalInput")
    b1_t = nc.dram_tensor("b1_t", [P, HC], F32, kind="ExternalInput")
    b2_r = nc.dram_tensor("b2_r", [1, OUT], F16, kind="ExternalInput")
    out_d = nc.dram_tensor("out", [NBT, P, OUT], F32, kind="ExternalOutput")

    with tile.TileContext(nc) as tc:
        small = tc.alloc_tile_pool(name="small", bufs=1)
        wpool = tc.alloc_tile_pool(name="wpool", bufs=1)
        w1p = tc.alloc_tile_pool(name="w1p", bufs=1)

        ident = small.tile([P, P], F16, tag="ident")
        make_identity(nc, ident[:])
        ones1 = small.tile([1, P], F16, tag="ones1")
        nc.vector.memset(ones1[:], 1.0)
        b1t = small.tile([P, HC], F32, tag="b1")
        nc.sync.dma_start(b1t[:], b1_t[:])
        b2t = small.tile([1, OUT], F16, tag="b2")
        nc.sync.dma_start(b2t[:], b2_r[:])
        xh = small.tile([P, DC, B_CORE], F16, tag="xh")
        nc.sync.dma_start(xh[:], xhT_d.rearrange("d p b -> p d b"))

        w = [wpool.tile([P, M], F16, tag=f"w{bt}", name=f"w{bt}")
             for bt in range(NBT)]
        w1t = [w1p.tile([P, H], F16, tag=f"w1_{dc}", name=f"w1_{dc}")
               for dc in range(DC)]

        st = {}
        for bt in range(NBT):
            d = {}
            d["mx"] = small.tile([P, MB], F32, tag=f"mx{bt}", name=f"mx{bt}")
            d["zsum"] = small.tile([P, MB], F32, tag=f"zs{bt}", name=f"zs{bt}")
            for nm in ["rm", "cap", "zsr", "sact", "gacc", "kv", "kg", "rk",
                       "sv", "step"]:
                d[nm] = small.tile([P, 1], F32, tag=f"{nm}{bt}", name=f"{nm}{bt}")
            d["tau"] = [small.tile([P, 1], F32, tag=f"tau{bt}_{r}",
                                   name=f"tau{bt}_{r}")
                        for r in range(N_ROUNDS + 1)]
            d["ntau"] = [small.tile([P, 1], F32, tag=f"ntau{bt}_{r}",
                                    name=f"ntau{bt}_{r}")
                         for r in range(N_ROUNDS + 1)]
            st[bt] = d

        # ---- persistent row tiles (released after w materialization) ----
        zpool = tc.alloc_tile_pool(name="zpool", bufs=1)
        z = [zpool.tile([P, M], F16, tag=f"z{bt}", name=f"z{bt}")
             for bt in range(NBT)]
        scr = [zpool.tile([P, M], F16, tag=f"scr{bt}", name=f"scr{bt}")
               for bt in range(NBT)]

        # ---- dist: z[bt] [P, M] fp16 + block sums/maxes ----
        mstream = tc.alloc_tile_pool(name="mstream", bufs=2)
        ps_dist = tc.alloc_tile_pool(name="ps_dist", bufs=4, space="PSUM")
        for blk in range(MB // 2):
            mt = mstream.tile([P, DC, 1024], F16, tag="memT")
            for dq in range(4):
                nc.sync.dma_start(
                    mt[:, dq * 2:(dq + 1) * 2],
                    memT[dq * 2:(dq + 1) * 2, :, blk * 1024:(blk + 1) * 1024]
                    .rearrange("d p m -> p d m"))
            for mh in range(2):
                mb = blk * 2 + mh
                for bt in range(NBT):
                    zp = ps_dist.tile([P, 512], F32, tag="zp")
                    for dc in range(DC):
                        nc.tensor.matmul(
                            zp[:], xh[:, dc, bt * P:(bt + 1) * P],
                            mt[:, dc, mh * 512:(mh + 1) * 512],
                            start=(dc == 0), stop=(dc == DC - 1))
                    nc.scalar.activation(
                        z[bt][:, mb * 512:(mb + 1) * 512], zp[:], AF.Copy,
                        accum_out=st[bt]["zsum"][:, mb:mb + 1])
                    nc.vector.reduce_max(
                        st[bt]["mx"][:, mb:mb + 1],
                        z[bt][:, mb * 512:(mb + 1) * 512], axis=AX.X)
        ps_dist.release()

        # W1 on the gpsimd DMA queue: transfers run during the sparsemax
        # window without blocking the sync queue's memR ring.
        for dc in range(DC):
            nc.gpsimd.dma_start(w1t[dc][:], w1c[dc])

        # ---- sparsemax init: tau0 = mean + T0_SIG, capped below rowmax ----
        for bt in range(NBT):
            d = st[bt]
            nc.vector.reduce_sum(d["zsr"][:], d["zsum"][:], axis=AX.X)
            nc.vector.reduce_max(d["rm"][:], d["mx"][:], axis=AX.X)
            nc.vector.tensor_scalar_add(d["cap"][:], d["rm"][:], -CAP_OFF)
            nc.vector.tensor_scalar(
                out=d["step"][:], in0=d["zsr"][:], scalar1=1.0 / M,
                scalar2=T0_SIG, op0=ALU.mult, op1=ALU.add)
            nc.vector.tensor_tensor(
                d["tau"][0][:], d["step"][:], d["cap"][:], ALU.min)
            nc.vector.tensor_scalar_mul(d["ntau"][0][:], d["tau"][0][:], -1.0)

        ps_warm = tc.alloc_tile_pool(name="ps_warm", bufs=2, space="PSUM")

        # ---- Newton rounds: S(tau), k(tau) -> tau += (S-1)/k ----
        for r in range(N_ROUNDS):
            for bt in range(NBT):
                d = st[bt]
                nc.scalar.activation(
                    w[bt][:, 0:MA], z[bt][:, 0:MA], AF.Relu,
                    bias=d["ntau"][r][:, 0:1], accum_out=d["sact"][:])
            for bt in range(NBT):
                d = st[bt]
                tau_s = d["tau"][r][:, 0:1]
                # z - tau first (fp16 error vanishes near zero, where the
                # support lives), then relu with fused sum accumulation
                nc.vector.tensor_scalar(
                    out=scr[bt][:, MA:M], in0=z[bt][:, MA:M],
                    scalar1=tau_s, scalar2=None, op0=ALU.subtract)
                nc.vector.tensor_scalar(
                    out=w[bt][:, MA:M], in0=scr[bt][:, MA:M],
                    scalar1=0.0, scalar2=None,
                    op0=ALU.max, op1=ALU.add, accum_out=d["gacc"][:])
                nc.vector.tensor_scalar(
                    out=scr[bt][:, 0:M], in0=z[bt][:, 0:M],
                    scalar1=tau_s, scalar2=None,
                    op0=ALU.is_ge, op1=ALU.add, accum_out=d["kv"][:])
                # scalar update chain
                nc.vector.tensor_add(d["sv"][:], d["sact"][:], d["gacc"][:])
                nc.vector.tensor_scalar_max(d["kg"][:], d["kv"][:], 1.0)
                nc.vector.reciprocal(d["rk"][:], d["kg"][:])
                nc.vector.tensor_scalar(
                    out=d["step"][:], in0=d["sv"][:], scalar1=-1.0,
                    scalar2=d["rk"][:, 0:1], op0=ALU.add, op1=ALU.mult)
                nc.vector.tensor_scalar(
                    out=d["tau"][r + 1][:], in0=d["step"][:],
                    scalar1=d["tau"][r][:, 0:1], scalar2=d["cap"][:, 0:1],
                    op0=ALU.add, op1=ALU.min)
                nc.vector.tensor_scalar_mul(
                    d["ntau"][r + 1][:], d["tau"][r + 1][:], -1.0)
            # keep the PE clock from dropping to the lowest p-state
            for bt in range(NBT):
                wp = ps_warm.tile([P, P], F16, tag="warm")
                nc.tensor.transpose(wp[:], scr[bt][:, 0:P], ident[:])

        # ---- final w materialization at converged tau ----
        for bt in range(NBT):
            d = st[bt]
            nf = d["ntau"][N_ROUNDS]
            nc.scalar.activation(
                w[bt][:, 0:MA], z[bt][:, 0:MA], AF.Relu, bias=nf[:, 0:1])
        for bt in range(NBT):
            d = st[bt]
            tau_s = d["tau"][N_ROUNDS][:, 0:1]
            nc.vector.tensor_scalar(
                out=scr[bt][:, MA:M], in0=z[bt][:, MA:M],
                scalar1=tau_s, scalar2=None, op0=ALU.subtract)
            nc.vector.tensor_scalar(
                out=w[bt][:, MA:M], in0=scr[bt][:, MA:M],
                scalar1=0.0, scalar2=None, op0=ALU.max)
        ps_warm.release()
        mstream.release()
        zpool.release()

        # ---- w^T transposes + memory read: mv[bt] = w[bt] @ memR ----
        wTt = tc.alloc_tile_pool(name="wTt", bufs=12)
        mring = tc.alloc_tile_pool(name="mring", bufs=RING)
        w2s = tc.alloc_tile_pool(name="w2s", bufs=W2BUF)
        ps_tr = tc.alloc_tile_pool(name="ps_tr", bufs=4, space="PSUM")
        ps_mv = tc.alloc_tile_pool(name="ps_mv", bufs=1, space="PSUM")

        # prefill the ring (these transfers run during the sparsemax window)
        slabs = []
        for mc in range(MC):
            slab = mring.tile([P, D], F16, tag="memR", name=f"memR{mc}")
            slabs.append(slab)
            if mc < RING:
                nc.sync.dma_start(slab[:], memR[mc])
        # W2 on the gpsimd queue (ring-gated; nothing vital queued behind)
        w2t = [w2s.tile([P, OUT], F16, tag="w2", name=f"w2_{kc}")
               for kc in range(HC)]
        for kc in range(HC):
            nc.gpsimd.dma_start(w2t[kc][:], w2c[kc])

        mv_ps = [[ps_mv.tile([P, 512], F32, tag=f"mv{bt}_{dh}",
                             name=f"mv{bt}_{dh}")
                  for dh in range(2)] for bt in range(NBT)]
        for mc in range(MC):
            if mc >= RING:
                nc.sync.dma_start(slabs[mc][:], memR[mc])
            tp = ps_tr.tile([P, B_CORE], F16, tag="wtr")
            for bt in range(NBT):
                nc.tensor.transpose(
                    tp[:, bt * P:(bt + 1) * P],
                    w[bt][:, mc * P:(mc + 1) * P], ident[:])
            wT = wTt.tile([P, B_CORE], F16, tag="wT")
            if mc % 2 == 0:
                nc.vector.tensor_copy(wT[:], tp[:])
            else:
                nc.scalar.copy(wT[:], tp[:])
            for bt in range(NBT):
                for dh in range(2):
                    nc.tensor.matmul(
                        mv_ps[bt][dh][:], wT[:, bt * P:(bt + 1) * P],
                        slabs[mc][:, dh * 512:(dh + 1) * 512],
                        start=(mc == 0), stop=(mc == MC - 1))

        # ---- mv evac (fp16) + transpose to mvT [P, dc, 256] ----
        mv_sb = [small.tile([P, D], F16, tag=f"mvsb{bt}", name=f"mvsb{bt}")
                 for bt in range(NBT)]
        for bt in range(NBT):
            for dh in range(2):
                nc.scalar.copy(mv_sb[bt][:, dh * 512:(dh + 1) * 512],
                               mv_ps[bt][dh][:])
        ps_mv.release()
        mvT = small.tile([P, DC, B_CORE], F16, tag="mvT")
        for dc in range(DC):
            tp = ps_tr.tile([P, B_CORE], F16, tag="wtr")
            for bt in range(NBT):
                nc.tensor.transpose(
                    tp[:, bt * P:(bt + 1) * P],
                    mv_sb[bt][:, dc * P:(dc + 1) * P], ident[:])
            if dc % 2 == 0:
                nc.vector.tensor_copy(mvT[:, dc], tp[:])
            else:
                nc.scalar.copy(mvT[:, dc], tp[:])

        # ---- MLP1: h[bt] [P(b), H] = mvT-blocks^T @ W1-chunks (bias later) --
        hsb = [small.tile([P, H], F16, tag=f"h{bt}", name=f"h{bt}")
               for bt in range(NBT)]
        ps_h = tc.alloc_tile_pool(name="ps_h", bufs=1, space="PSUM")
        hps = [ps_h.tile([P, 512], F32, tag=f"hp{hb}", name=f"hp{hb}")
               for hb in range(HB)]
        for bt in range(NBT):
            for dc in range(DC):
                for hb in range(HB):
                    nc.tensor.matmul(
                        hps[hb][:], mvT[:, dc, bt * P:(bt + 1) * P],
                        w1t[dc][:, hb * 512:(hb + 1) * 512],
                        start=(dc == 0), stop=(dc == DC - 1))
            for hb in range(HB):
                nc.scalar.copy(hsb[bt][:, hb * 512:(hb + 1) * 512], hps[hb][:])
        ps_h.release()

        # ---- hT transposes; relu + b1 fused into the per-partition evac ----
        hT = small.tile([P, HC, B_CORE], F16, tag="hT")
        for hc in range(HC):
            tp = ps_tr.tile([P, B_CORE], F16, tag="wtr")
            for bt in range(NBT):
                nc.tensor.transpose(
                    tp[:, bt * P:(bt + 1) * P],
                    hsb[bt][:, hc * P:(hc + 1) * P], ident[:])
            nc.scalar.activation(
                hT[:, hc], tp[:], AF.Relu, bias=b1t[:, hc:hc + 1])

        # ---- MLP2: out[bt] = hT-blocks^T @ W2 + b2 ----
        ps_o = tc.alloc_tile_pool(name="ps_o", bufs=1, space="PSUM")
        osb = [small.tile([P, OUT], F32, tag=f"osb{bt}", name=f"osb{bt}")
               for bt in range(NBT)]
        ops = [[ps_o.tile([P, NW], F32, tag=f"op{bt}_{oh}",
                          name=f"op{bt}_{oh}")
                for oh in range(NOH)] for bt in range(NBT)]
        for kc in range(HC):
            for bt in range(NBT):
                for oh in range(NOH):
                    nc.tensor.matmul(
                        ops[bt][oh][:], hT[:, kc, bt * P:(bt + 1) * P],
                        w2t[kc][:, oh * NW:(oh + 1) * NW],
                        start=(kc == 0), stop=False)
        for bt in range(NBT):
            for oh in range(NOH):
                nc.tensor.matmul(
                    ops[bt][oh][:], ones1[:], b2t[:, oh * NW:(oh + 1) * NW],
                    start=False, stop=True)
                nc.scalar.copy(osb[bt][:, oh * NW:(oh + 1) * NW],
                               ops[bt][oh][:])
            nc.sync.dma_start(out_d[bt], osb[bt][:])
        ps_o.release()
        ps_tr.release()
        w2s.release()
        mring.release()
        wTt.release()
        w1p.release()
        wpool.release()
        small.release()

    nc.compile()
    return nc


_CACHED = None


def _prep(inputs):
    x = np.ascontiguousarray(inputs["encoder_output"], dtype=np.float32)
    mem = np.ascontiguousarray(inputs["memory_set"], dtype=np.float32)
    W1 = np.ascontiguousarray(inputs["W1"], dtype=np.float32)
    b1 = np.ascontiguousarray(inputs["b1"], dtype=np.float32)
    W2 = np.ascontiguousarray(inputs["W2"], dtype=np.float32)
    b2 = np.ascontiguousarray(inputs["b2"], dtype=np.float32)

    inv_nx = 1.0 / np.sqrt((x * x).sum(1) + _EPS)
    inv_ny = 1.0 / np.sqrt((mem * mem).sum(1) + _EPS)
    xh = (x * inv_nx[:, None]).astype(np.float16)
    memT_hat = np.ascontiguousarray(
        (mem.T * inv_ny[None, :]).astype(np.float16).reshape(DC, P, M))
    memR_v = np.ascontiguousarray(mem.astype(np.float16).reshape(MC, P, D))
    w1_blk = np.ascontiguousarray(W1.astype(np.float16).reshape(DC, P, H))
    w2_blk = np.ascontiguousarray(W2.astype(np.float16).reshape(HC, P, OUT))
    b1_tiles = np.ascontiguousarray(b1.reshape(HC, P).T.astype(np.float32))
    b2_row = np.ascontiguousarray(b2.reshape(1, OUT).astype(np.float16))

    shared = {
        "memT": memT_hat, "memR": memR_v, "w1c": w1_blk,
        "w2c": w2_blk, "b1_t": b1_tiles, "b2_r": b2_row,
    }
    in_maps = []
    for c in range(8):
        xc = xh[c * B_CORE:(c + 1) * B_CORE]          # [256, 1024]
        xhT = np.ascontiguousarray(xc.T.reshape(DC, P, B_CORE))
        in_maps.append({"xhT": xhT, **shared})
    return in_maps


def kernel(**inputs) -> np.ndarray:
    global _CACHED
    if _CACHED is None:
        _CACHED = build()
    nc = _CACHED
    in_maps = _prep(inputs)
    res = run_bass_kernel_spmd(nc, in_maps, core_ids=list(range(8)))
    return np.concatenate(
        [r["out"].reshape(B_CORE, OUT) for r in res.results], axis=0)


# revision 10
# speedup vs baseline: 1.3355x; 1.2530x over previous
"""Trainium2 Bass kernel for nn_BaselineMemory (sparse attention memory read + MLP).

Data-parallel over batch: each of 8 NeuronCores handles 256 of 2048 rows.
fp16 end-to-end (fp32 PSUM/accum). Host pre-normalizes x and mem^T.
Pipeline per core:
  dist matmul z = x_hat @ y_hat^T (fp16, PE) -> z fp16 + block sums/maxes
  -> sparsemax tau via 3 Newton rounds from a Gaussian-moment init
     (S(tau) = ACT relu head + DVE tail; support count k on DVE is_ge;
      tau += (S-1)/k) -> w materialization pass
  -> w^T transposes (PE, fp16) -> memory read mv^T (fp16, PE)
  -> MLP1 (W1 natural layout, relu+b1 fused into post-transpose evac)
  -> MLP2 (+b2 via rank-1 matmul) -> fp32 out.
DMA: memT stream + memR ring + outputs on the Sync queue; W1/W2 on the
GpSimd queue so they transfer during the sparsemax window.
"""
import sys

if "/opt/trn_rl_repo" not in sys.path:
    sys.path.insert(0, "/opt/trn_rl_repo")

import numpy as np

import concourse.bass as bass  # noqa: F401
import concourse.tile as tile
from concourse import bacc, mybir
from concourse.bass_utils import run_bass_kernel_spmd
from concourse.masks import make_identity

P = 128
B_CORE = 256          # batch rows per core
NBT = B_CORE // P     # 2 b-tiles
D = 1024
DC = D // P           # 8 d-chunks
M = 8192
MB = M // 512         # 16 dist m-blocks per bt
MC = M // P           # 64 m-chunks for read
H = 2048
HC = H // P           # 16 h-chunks
HB = H // 512         # 4 mlp1 col-blocks
OUT = 1000
NOH = 2               # out halves of 500
NW = OUT // NOH

N_ROUNDS = 3          # quasi-Newton iterations on tau
MA = 5632             # S-pass head handled by ACT; tail [MA, M) on DVE
TAIL = M - MA
T0_SIG = 2.25 / 32.0  # init: tau0 = mean + 2.25*sigma, sigma = 1/sqrt(d)
INV_S2 = 22.627417    # 1/(sigma*sqrt(2)) = 32/sqrt(2)
KHALF = 4096.0        # m/2 for the erfc slope model
CLIP = 1.0 / 16.0     # step clip (2*sigma)
CAP_OFF = 1e-4
RING = 24             # memR ring slabs resident
W2BUF = 8
LAG = 8               # read matmuls trail w^T transposes by LAG chunks

F32 = mybir.dt.float32
F16 = mybir.dt.float16
AF = mybir.ActivationFunctionType
ALU = mybir.AluOpType
AX = mybir.AxisListType

_EPS = 1e-6


def build():
    nc = bacc.Bacc("TRN2", target_bir_lowering=False, debug=False)

    xhT_d = nc.dram_tensor("xhT", [DC, P, B_CORE], F16, kind="ExternalInput")
    memT = nc.dram_tensor("memT", [DC, P, M], F16, kind="ExternalInput")
    memR = nc.dram_tensor("memR", [MC, P, D], F16, kind="ExternalInput")
    w1c = nc.dram_tensor("w1c", [DC, P, H], F16, kind="ExternalInput")
    w2c = nc.dram_tensor("w2c", [HC, P, OUT], F16, kind="ExternalInput")
    b1_t = nc.dram_tensor("b1_t", [P, HC], F32, kind="ExternalInput")
    b2_r = nc.dram_tensor("b2_r", [1, OUT], F16, kind="ExternalInput")
    out_d = nc.dram_tensor("out", [NBT, P, OUT], F32, kind="ExternalOutput")

    with tile.TileContext(nc) as tc:
        small = tc.alloc_tile_pool(name="small", bufs=1)
        wpool = tc.alloc_tile_pool(name="wpool", bufs=1)
        w1p = tc.alloc_tile_pool(name="w1p", bufs=1)

        ident = small.tile([P, P], F16, tag="ident")
        make_identity(nc, ident[:])
        ones1 = small.tile([1, P], F16, tag="ones1")
        nc.vector.memset(ones1[:], 1.0)
        b1t = small.tile([P, HC], F32, tag="b1")
        nc.sync.dma_start(b1t[:], b1_t[:])
        b2t = small.tile([1, OUT], F16, tag="b2")
        nc.sync.dma_start(b2t[:], b2_r[:])
        xh = small.tile([P, DC, B_CORE], F16, tag="xh")
        nc.sync.dma_start(xh[:], xhT_d.rearrange("d p b -> p d b"))

        w = [wpool.tile([P, M], F16, tag=f"w{bt}", name=f"w{bt}")
             for bt in range(NBT)]
        w1t = [w1p.tile([P, H], F16, tag=f"w1_{dc}", name=f"w1_{dc}")
               for dc in range(DC)]

        st = {}
        for bt in range(NBT):
            d = {}
            d["mx"] = small.tile([P, MB], F32, tag=f"mx{bt}", name=f"mx{bt}")
            d["zsum"] = small.tile([P, MB], F32, tag=f"zs{bt}", name=f"zs{bt}")
            for nm in ["rm", "cap", "zsr", "mu", "sact", "gacc", "targ",
                       "erf", "kg", "rk", "sv", "step", "stepc"]:
                d[nm] = small.tile([P, 1], F32, tag=f"{nm}{bt}", name=f"{nm}{bt}")
            d["tau"] = [small.tile([P, 1], F32, tag=f"tau{bt}_{r}",
                                   name=f"tau{bt}_{r}")
                        for r in range(N_ROUNDS + 1)]
            d["ntau"] = [small.tile([P, 1], F32, tag=f"ntau{bt}_{r}",
                                    name=f"ntau{bt}_{r}")
                         for r in range(N_ROUNDS + 1)]
            st[bt] = d

        # ---- persistent row tiles (released after w materialization) ----
        zpool = tc.alloc_tile_pool(name="zpool", bufs=1)
        z = [zpool.tile([P, M], F16, tag=f"z{bt}", name=f"z{bt}")
             for bt in range(NBT)]
        scr = [zpool.tile([P, TAIL], F16, tag=f"scr{bt}", name=f"scr{bt}")
               for bt in range(NBT)]

        # ---- PE warmup burst: ramp the clock while the first DMAs land ----
        junk = small.tile([P, 512], F16, tag="junk")
        nc.vector.memset(junk[:], 1.0)

        # ---- dist: z[bt] [P, M] fp16 + block sums/maxes ----
        mstream = tc.alloc_tile_pool(name="mstream", bufs=2)
        ps_dist = tc.alloc_tile_pool(name="ps_dist", bufs=4, space="PSUM")
        for i in range(10):
            wup = ps_dist.tile([P, 512], F32, tag="zp")
            nc.tensor.matmul(wup[:], ident[:], junk[:], start=True, stop=True)
        for blk in range(MB // 2):
            mt = mstream.tile([P, DC, 1024], F16, tag="memT")
            for dq in range(4):
                nc.sync.dma_start(
                    mt[:, dq * 2:(dq + 1) * 2],
                    memT[dq * 2:(dq + 1) * 2, :, blk * 1024:(blk + 1) * 1024]
                    .rearrange("d p m -> p d m"))
            for mh in range(2):
                mb = blk * 2 + mh
                for bt in range(NBT):
                    zp = ps_dist.tile([P, 512], F32, tag="zp")
                    for dc in range(DC):
                        nc.tensor.matmul(
                            zp[:], xh[:, dc, bt * P:(bt + 1) * P],
                            mt[:, dc, mh * 512:(mh + 1) * 512],
                            start=(dc == 0), stop=(dc == DC - 1))
                    nc.scalar.activation(
                        z[bt][:, mb * 512:(mb + 1) * 512], zp[:], AF.Copy,
                        accum_out=st[bt]["zsum"][:, mb:mb + 1])
                    nc.vector.reduce_max(
                        st[bt]["mx"][:, mb:mb + 1],
                        z[bt][:, mb * 512:(mb + 1) * 512], axis=AX.X)
        ps_dist.release()

        # W1 on the gpsimd DMA queue: transfers run during the sparsemax
        # window without blocking the sync queue's memR ring.
        for dc in range(DC):
            nc.gpsimd.dma_start(w1t[dc][:], w1c[dc])

        # ---- sparsemax init: tau0 = mean + T0_SIG, capped below rowmax ----
        for bt in range(NBT):
            d = st[bt]
            nc.vector.reduce_sum(d["zsr"][:], d["zsum"][:], axis=AX.X)
            nc.vector.reduce_max(d["rm"][:], d["mx"][:], axis=AX.X)
            nc.vector.tensor_scalar_add(d["cap"][:], d["rm"][:], -CAP_OFF)
            nc.vector.tensor_scalar_mul(d["mu"][:], d["zsr"][:], 1.0 / M)
            nc.vector.tensor_scalar_add(d["step"][:], d["mu"][:], T0_SIG)
            nc.vector.tensor_tensor(
                d["tau"][0][:], d["step"][:], d["cap"][:], ALU.min)
            nc.vector.tensor_scalar_mul(d["ntau"][0][:], d["tau"][0][:], -1.0)

        ps_warm = tc.alloc_tile_pool(name="ps_warm", bufs=2, space="PSUM")

        # ---- quasi-Newton rounds: S(tau) measured, slope from the
        # Gaussian model k = m/2*erfc((tau-mu)/(sigma*sqrt2)) ----
        for r in range(N_ROUNDS):
            for bt in range(NBT):
                # slope model: depends only on tau[r] -> off the critical path
                d = st[bt]
                nc.vector.tensor_scalar(
                    out=d["targ"][:], in0=d["tau"][r][:], scalar1=d["mu"][:, 0:1],
                    scalar2=INV_S2, op0=ALU.subtract, op1=ALU.mult)
            for bt in range(NBT):
                d = st[bt]
                nc.scalar.activation(
                    w[bt][:, 0:MA], z[bt][:, 0:MA], AF.Relu,
                    bias=d["ntau"][r][:, 0:1], accum_out=d["sact"][:])
                nc.scalar.activation(d["erf"][:], d["targ"][:], AF.Erf)
            for bt in range(NBT):
                d = st[bt]
                nc.vector.tensor_scalar(
                    out=d["kg"][:], in0=d["erf"][:], scalar1=-KHALF,
                    scalar2=KHALF, op0=ALU.mult, op1=ALU.add)
                nc.vector.tensor_scalar_max(d["kg"][:], d["kg"][:], 1.0)
                nc.vector.reciprocal(d["rk"][:], d["kg"][:])
            for bt in range(NBT):
                d = st[bt]
                tau_s = d["tau"][r][:, 0:1]
                # z - tau first (fp16 error vanishes near zero, where the
                # support lives), then relu with fused sum accumulation
                nc.vector.tensor_scalar(
                    out=scr[bt][:], in0=z[bt][:, MA:M],
                    scalar1=tau_s, scalar2=None, op0=ALU.subtract)
                nc.vector.tensor_scalar(
                    out=w[bt][:, MA:M], in0=scr[bt][:],
                    scalar1=0.0, scalar2=None,
                    op0=ALU.max, op1=ALU.add, accum_out=d["gacc"][:])
                nc.vector.tensor_add(d["sv"][:], d["sact"][:], d["gacc"][:])
                nc.vector.tensor_scalar(
                    out=d["step"][:], in0=d["sv"][:], scalar1=-1.0,
                    scalar2=d["rk"][:, 0:1], op0=ALU.add, op1=ALU.mult)
                nc.vector.tensor_scalar(
                    out=d["stepc"][:], in0=d["step"][:], scalar1=CLIP,
                    scalar2=-CLIP, op0=ALU.min, op1=ALU.max)
                nc.vector.tensor_scalar(
                    out=d["tau"][r + 1][:], in0=d["stepc"][:],
                    scalar1=d["tau"][r][:, 0:1], scalar2=d["cap"][:, 0:1],
                    op0=ALU.add, op1=ALU.min)
                nc.vector.tensor_scalar_mul(
                    d["ntau"][r + 1][:], d["tau"][r + 1][:], -1.0)
            # keep the PE clock from dropping to the lowest p-state
            for bt in range(NBT):
                wp = ps_warm.tile([P, P], F16, tag="warm")
                nc.tensor.transpose(wp[:], scr[bt][:, 0:P], ident[:])

        # ---- final w materialization at converged tau ----
        for bt in range(NBT):
            d = st[bt]
            nf = d["ntau"][N_ROUNDS]
            nc.scalar.activation(
                w[bt][:, 0:MA], z[bt][:, 0:MA], AF.Relu, bias=nf[:, 0:1])
        for bt in range(NBT):
            d = st[bt]
            tau_s = d["tau"][N_ROUNDS][:, 0:1]
            nc.vector.tensor_scalar(
                out=scr[bt][:], in0=z[bt][:, MA:M],
                scalar1=tau_s, scalar2=None, op0=ALU.subtract)
            nc.vector.tensor_scalar(
                out=w[bt][:, MA:M], in0=scr[bt][:],
                scalar1=0.0, scalar2=None, op0=ALU.max)
        ps_warm.release()
        mstream.release()
        zpool.release()

        # ---- w^T transposes + memory read: mv[bt] = w[bt] @ memR ----
        wTt = tc.alloc_tile_pool(name="wTt", bufs=12)
        mring = tc.alloc_tile_pool(name="mring", bufs=RING)
        w2s = tc.alloc_tile_pool(name="w2s", bufs=W2BUF)
        ps_tr = tc.alloc_tile_pool(name="ps_tr", bufs=4, space="PSUM")
        ps_mv = tc.alloc_tile_pool(name="ps_mv", bufs=1, space="PSUM")

        # prefill the ring (these transfers run during the sparsemax window)
        slabs = []
        for mc in range(MC):
            slab = mring.tile([P, D], F16, tag="memR", name=f"memR{mc}")
            slabs.append(slab)
            if mc < RING:
                nc.sync.dma_start(slab[:], memR[mc])
        # W2 on the gpsimd queue (ring-gated; nothing vital queued behind)
        w2t = [w2s.tile([P, OUT], F16, tag="w2", name=f"w2_{kc}")
               for kc in range(HC)]
        for kc in range(HC):
            nc.gpsimd.dma_start(w2t[kc][:], w2c[kc])

        mv_ps = [[ps_mv.tile([P, 512], F32, tag=f"mv{bt}_{dh}",
                             name=f"mv{bt}_{dh}")
                  for dh in range(2)] for bt in range(NBT)]
        # transposes run LAG chunks ahead of the read matmuls so the PE never
        # stalls on the cross-engine psum->sbuf evacuation roundtrip
        wTs = []
        for it in range(MC + LAG):
            if it < MC:
                mc = it
                if mc >= RING:
                    nc.sync.dma_start(slabs[mc][:], memR[mc])
                tp = ps_tr.tile([P, B_CORE], F16, tag="wtr")
                for bt in range(NBT):
                    nc.tensor.transpose(
                        tp[:, bt * P:(bt + 1) * P],
                        w[bt][:, mc * P:(mc + 1) * P], ident[:])
                wT = wTt.tile([P, B_CORE], F16, tag="wT", name=f"wT{mc}")
                wTs.append(wT)
                if mc % 2 == 0:
                    nc.vector.tensor_copy(wT[:], tp[:])
                else:
                    nc.scalar.copy(wT[:], tp[:])
            if it >= LAG:
                mc = it - LAG
                for bt in range(NBT):
                    for dh in range(2):
                        nc.tensor.matmul(
                            mv_ps[bt][dh][:], wTs[mc][:, bt * P:(bt + 1) * P],
                            slabs[mc][:, dh * 512:(dh + 1) * 512],
                            start=(mc == 0), stop=(mc == MC - 1))

        # ---- mv evac (fp16) + transpose to mvT [P, dc, 256] ----
        mv_sb = [small.tile([P, D], F16, tag=f"mvsb{bt}", name=f"mvsb{bt}")
                 for bt in range(NBT)]
        for bt in range(NBT):
            for dh in range(2):
                nc.scalar.copy(mv_sb[bt][:, dh * 512:(dh + 1) * 512],
                               mv_ps[bt][dh][:])
        ps_mv.release()
        mvT = small.tile([P, DC, B_CORE], F16, tag="mvT")
        for dc in range(DC):
            tp = ps_tr.tile([P, B_CORE], F16, tag="wtr")
            for bt in range(NBT):
                nc.tensor.transpose(
                    tp[:, bt * P:(bt + 1) * P],
                    mv_sb[bt][:, dc * P:(dc + 1) * P], ident[:])
            if dc % 2 == 0:
                nc.vector.tensor_copy(mvT[:, dc], tp[:])
            else:
                nc.scalar.copy(mvT[:, dc], tp[:])

        # ---- MLP1: h[bt] [P(b), H] = mvT-blocks^T @ W1-chunks (bias later) --
        hsb = [small.tile([P, H], F16, tag=f"h{bt}", name=f"h{bt}")
               for bt in range(NBT)]
        ps_h = tc.alloc_tile_pool(name="ps_h", bufs=1, space="PSUM")
        hps = [ps_h.tile([P, 512], F32, tag=f"hp{hb}", name=f"hp{hb}")
               for hb in range(HB)]
        for bt in range(NBT):
            for dc in range(DC):
                for hb in range(HB):
                    nc.tensor.matmul(
                        hps[hb][:], mvT[:, dc, bt * P:(bt + 1) * P],
                        w1t[dc][:, hb * 512:(hb + 1) * 512],
                        start=(dc == 0), stop=(dc == DC - 1))
            for hb in range(HB):
                nc.scalar.copy(hsb[bt][:, hb * 512:(hb + 1) * 512], hps[hb][:])
        ps_h.release()

        # ---- hT transposes; relu + b1 fused into the per-partition evac ----
        hT = small.tile([P, HC, B_CORE], F16, tag="hT")
        for hc in range(HC):
            tp = ps_tr.tile([P, B_CORE], F16, tag="wtr")
            for bt in range(NBT):
                nc.tensor.transpose(
                    tp[:, bt * P:(bt + 1) * P],
                    hsb[bt][:, hc * P:(hc + 1) * P], ident[:])
            nc.scalar.activation(
                hT[:, hc], tp[:], AF.Relu, bias=b1t[:, hc:hc + 1])

        # ---- MLP2: out[bt] = hT-blocks^T @ W2 + b2 ----
        ps_o = tc.alloc_tile_pool(name="ps_o", bufs=1, space="PSUM")
        osb = [small.tile([P, OUT], F32, tag=f"osb{bt}", name=f"osb{bt}")
               for bt in range(NBT)]
        ops = [[ps_o.tile([P, NW], F32, tag=f"op{bt}_{oh}",
                          name=f"op{bt}_{oh}")
                for oh in range(NOH)] for bt in range(NBT)]
        for kc in range(HC):
            for bt in range(NBT):
                for oh in range(NOH):
                    nc.tensor.matmul(
                        ops[bt][oh][:], hT[:, kc, bt * P:(bt + 1) * P],
                        w2t[kc][:, oh * NW:(oh + 1) * NW],
                        start=(kc == 0), stop=False)
        for bt in range(NBT):
            for oh in range(NOH):
                nc.tensor.matmul(
                    ops[bt][oh][:], ones1[:], b2t[:, oh * NW:(oh + 1) * NW],
                    start=False, stop=True)
                nc.scalar.copy(osb[bt][:, oh * NW:(oh + 1) * NW],
                               ops[bt][oh][:])
            nc.sync.dma_start(out_d[bt], osb[bt][:])
        ps_o.release()
        ps_tr.release()
        w2s.release()
        mring.release()
        wTt.release()
        w1p.release()
        wpool.release()
        small.release()

    nc.compile()
    return nc


_CACHED = None


def _prep(inputs):
    x = np.ascontiguousarray(inputs["encoder_output"], dtype=np.float32)
    mem = np.ascontiguousarray(inputs["memory_set"], dtype=np.float32)
    W1 = np.ascontiguousarray(inputs["W1"], dtype=np.float32)
    b1 = np.ascontiguousarray(inputs["b1"], dtype=np.float32)
    W2 = np.ascontiguousarray(inputs["W2"], dtype=np.float32)
    b2 = np.ascontiguousarray(inputs["b2"], dtype=np.float32)

    inv_nx = 1.0 / np.sqrt((x * x).sum(1) + _EPS)
    inv_ny = 1.0 / np.sqrt((mem * mem).sum(1) + _EPS)
    xh = (x * inv_nx[:, None]).astype(np.float16)
    memT_hat = np.ascontiguousarray(
        (mem.T * inv_ny[None, :]).astype(np.float16).reshape(DC, P, M))
    memR_v = np.ascontiguousarray(mem.astype(np.float16).reshape(MC, P, D))
    w1_blk = np.ascontiguousarray(W1.astype(np.float16).reshape(DC, P, H))
    w2_blk = np.ascontiguousarray(W2.astype(np.float16).reshape(HC, P, OUT))
    b1_tiles = np.ascontiguousarray(b1.reshape(HC, P).T.astype(np.float32))
    b2_row = np.ascontiguousarray(b2.reshape(1, OUT).astype(np.float16))

    shared = {
        "memT": memT_hat, "memR": memR_v, "w1c": w1_blk,
        "w2c": w2_blk, "b1_t": b1_tiles, "b2_r": b2_row,
    }
    in_maps = []
    for c in range(8):
        xc = xh[c * B_CORE:(c + 1) * B_CORE]          # [256, 1024]
        xhT = np.ascontiguousarray(xc.T.reshape(DC, P, B_CORE))
        in_maps.append({"xhT": xhT, **shared})
    return in_maps


def kernel(**inputs) -> np.ndarray:
    global _CACHED
    if _CACHED is None:
        _CACHED = build()
    nc = _CACHED
    in_maps = _prep(inputs)
    res = run_bass_kernel_spmd(nc, in_maps, core_ids=list(range(8)))
    return np.concatenate(
        [r["out"].reshape(B_CORE, OUT) for r in res.results], axis=0)


# revision 13
# speedup vs baseline: 1.3713x; 1.0268x over previous
"""Trainium2 Bass kernel for nn_BaselineMemory (sparse attention memory read + MLP).

Data-parallel over batch: each of 8 NeuronCores handles 256 of 2048 rows.
fp16 end-to-end (fp32 PSUM/accum). Host pre-normalizes x and mem^T.
Pipeline per core:
  dist matmul z = x_hat @ y_hat^T (fp16, PE) -> z fp16 + block sums/maxes
  -> sparsemax tau via 3 Newton rounds from a Gaussian-moment init
     (S(tau) = ACT relu head + DVE tail; support count k on DVE is_ge;
      tau += (S-1)/k) -> w materialization pass
  -> w^T transposes (PE, fp16) -> memory read mv^T (fp16, PE)
  -> MLP1 (W1 natural layout, relu+b1 fused into post-transpose evac)
  -> MLP2 (+b2 via rank-1 matmul) -> fp32 out.
DMA: memT stream + memR ring + outputs on the Sync queue; W1/W2 on the
GpSimd queue so they transfer during the sparsemax window.
"""
import sys

if "/opt/trn_rl_repo" not in sys.path:
    sys.path.insert(0, "/opt/trn_rl_repo")

import numpy as np

import concourse.bass as bass  # noqa: F401
import concourse.tile as tile
from concourse import bacc, mybir
from concourse.bass_utils import run_bass_kernel_spmd
from concourse.masks import make_identity

P = 128
B_CORE = 256          # batch rows per core
NBT = B_CORE // P     # 2 b-tiles
D = 1024
DC = D // P           # 8 d-chunks
M = 8192
MB = M // 512         # 16 dist m-blocks per bt
MC = M // P           # 64 m-chunks for read
H = 2048
HC = H // P           # 16 h-chunks
HB = H // 512         # 4 mlp1 col-blocks
OUT = 1000
NOH = 2               # out halves of 500
NW = OUT // NOH

N_ROUNDS = 3          # quasi-Newton iterations on tau
MA = 5120             # S-pass head handled by ACT; tail [MA, M) on DVE
TAIL = M - MA
T0_SIG = 2.25 / 32.0  # init: tau0 = mean + 2.25*sigma, sigma = 1/sqrt(d)
INV_S2 = 22.627417    # 1/(sigma*sqrt(2)) = 32/sqrt(2)
KHALF = 4096.0        # m/2 for the erfc slope model
CLIP = 1.0 / 16.0     # step clip (2*sigma)
CAP_OFF = 1e-4
RING = 24             # memR ring slabs resident
W2BUF = 8
LAG = 8               # read matmuls trail w^T transposes by LAG chunks

F32 = mybir.dt.float32
F16 = mybir.dt.float16
AF = mybir.ActivationFunctionType
ALU = mybir.AluOpType
AX = mybir.AxisListType

_EPS = 1e-6


def build():
    nc = bacc.Bacc("TRN2", target_bir_lowering=False, debug=False)

    xhT_d = nc.dram_tensor("xhT", [P, DC * B_CORE], F16, kind="ExternalInput")
    memT = nc.dram_tensor("memT", [DC, P, M], F16, kind="ExternalInput")
    memR = nc.dram_tensor("memR", [MC, P, D], F16, kind="ExternalInput")
    w1c = nc.dram_tensor("w1c", [DC, P, H], F16, kind="ExternalInput")
    w2c = nc.dram_tensor("w2c", [HC, P, OUT], F16, kind="ExternalInput")
    b1_t = nc.dram_tensor("b1_t", [P, HC], F32, kind="ExternalInput")
    b2_r = nc.dram_tensor("b2_r", [1, OUT], F16, kind="ExternalInput")
    out_d = nc.dram_tensor("out", [NBT, P, OUT], F32, kind="ExternalOutput")

    with tile.TileContext(nc) as tc:
        small = tc.alloc_tile_pool(name="small", bufs=1)
        wpool = tc.alloc_tile_pool(name="wpool", bufs=1)
        w1p = tc.alloc_tile_pool(name="w1p", bufs=1)

        ident = small.tile([P, P], F16, tag="ident")
        make_identity(nc, ident[:])
        ones1 = small.tile([1, P], F16, tag="ones1")
        nc.vector.memset(ones1[:], 1.0)
        b1t = small.tile([P, HC], F32, tag="b1")
        nc.gpsimd.dma_start(b1t[:], b1_t[:])
        b2t = small.tile([1, OUT], F16, tag="b2")
        nc.gpsimd.dma_start(b2t[:], b2_r[:])
        xh = small.tile([P, DC * B_CORE], F16, tag="xh")
        nc.gpsimd.dma_start(xh[:], xhT_d[:])

        w = [wpool.tile([P, M], F16, tag=f"w{bt}", name=f"w{bt}")
             for bt in range(NBT)]
        w1t = [w1p.tile([P, H], F16, tag=f"w1_{dc}", name=f"w1_{dc}")
               for dc in range(DC)]

        st = {}
        for bt in range(NBT):
            d = {}
            d["mx"] = small.tile([P, MB], F32, tag=f"mx{bt}", name=f"mx{bt}")
            d["zsum"] = small.tile([P, MB], F32, tag=f"zs{bt}", name=f"zs{bt}")
            for nm in ["rm", "cap", "zsr", "mu", "sact", "gacc", "targ",
                       "erf", "kg", "rk", "sv", "step", "stepc"]:
                d[nm] = small.tile([P, 1], F32, tag=f"{nm}{bt}", name=f"{nm}{bt}")
            d["tau"] = [small.tile([P, 1], F32, tag=f"tau{bt}_{r}",
                                   name=f"tau{bt}_{r}")
                        for r in range(N_ROUNDS + 1)]
            d["ntau"] = [small.tile([P, 1], F32, tag=f"ntau{bt}_{r}",
                                    name=f"ntau{bt}_{r}")
                         for r in range(N_ROUNDS + 1)]
            st[bt] = d

        # ---- persistent row tiles (released after w materialization) ----
        zpool = tc.alloc_tile_pool(name="zpool", bufs=1)
        z = [zpool.tile([P, M], F16, tag=f"z{bt}", name=f"z{bt}")
             for bt in range(NBT)]
        scr = [zpool.tile([P, TAIL], F16, tag=f"scr{bt}", name=f"scr{bt}")
               for bt in range(NBT)]

        # ---- PE warmup burst: ramp the clock while the first DMAs land ----
        junk = small.tile([P, 512], F16, tag="junk")
        nc.vector.memset(junk[:], 1.0)

        # ---- dist: z[bt] [P, M] fp16 + block sums/maxes ----
        mstream = tc.alloc_tile_pool(name="mstream", bufs=2)
        ps_wu = tc.alloc_tile_pool(name="ps_wu", bufs=2, space="PSUM")
        for i in range(10):
            wup = ps_wu.tile([P, 512], F32, tag="wu")
            nc.tensor.matmul(wup[:], ident[:], junk[:], start=True, stop=True)
        ps_wu.release()
        ps_dist = tc.alloc_tile_pool(name="ps_dist", bufs=4, space="PSUM")
        for blk in range(MB // 2):
            mt = mstream.tile([P, DC, 1024], F16, tag="memT")
            for dq in range(4):
                nc.sync.dma_start(
                    mt[:, dq * 2:(dq + 1) * 2],
                    memT[dq * 2:(dq + 1) * 2, :, blk * 1024:(blk + 1) * 1024]
                    .rearrange("d p m -> p d m"))
            for mh in range(2):
                mb = blk * 2 + mh
                for bt in range(NBT):
                    zp = ps_dist.tile([P, 512], F32, tag="zp")
                    for dc in range(DC):
                        nc.tensor.matmul(
                            zp[:],
                            xh[:, dc * B_CORE + bt * P:
                               dc * B_CORE + (bt + 1) * P],
                            mt[:, dc, mh * 512:(mh + 1) * 512],
                            start=(dc == 0), stop=(dc == DC - 1))
                    nc.scalar.activation(
                        z[bt][:, mb * 512:(mb + 1) * 512], zp[:], AF.Copy,
                        accum_out=st[bt]["zsum"][:, mb:mb + 1])
                    nc.vector.reduce_max(
                        st[bt]["mx"][:, mb:mb + 1],
                        z[bt][:, mb * 512:(mb + 1) * 512], axis=AX.X)
        ps_dist.release()

        # W1 on the gpsimd DMA queue: transfers run during the sparsemax
        # window without blocking the sync queue's memR ring.
        for dc in range(DC):
            nc.gpsimd.dma_start(w1t[dc][:], w1c[dc])

        # ---- sparsemax init: tau0 = mean + T0_SIG, capped below rowmax ----
        for bt in range(NBT):
            d = st[bt]
            nc.vector.reduce_sum(d["zsr"][:], d["zsum"][:], axis=AX.X)
            nc.vector.reduce_max(d["rm"][:], d["mx"][:], axis=AX.X)
            nc.vector.tensor_scalar_add(d["cap"][:], d["rm"][:], -CAP_OFF)
            nc.vector.tensor_scalar_mul(d["mu"][:], d["zsr"][:], 1.0 / M)
            nc.vector.tensor_scalar_add(d["step"][:], d["mu"][:], T0_SIG)
            nc.vector.tensor_tensor(
                d["tau"][0][:], d["step"][:], d["cap"][:], ALU.min)
            nc.vector.tensor_scalar_mul(d["ntau"][0][:], d["tau"][0][:], -1.0)

        ps_warm = tc.alloc_tile_pool(name="ps_warm", bufs=2, space="PSUM")

        # ---- quasi-Newton rounds: S(tau) measured, slope from the
        # Gaussian model k = m/2*erfc((tau-mu)/(sigma*sqrt2)) ----
        for r in range(N_ROUNDS):
            for bt in range(NBT):
                # slope model: depends only on tau[r] -> off the critical path
                d = st[bt]
                nc.vector.tensor_scalar(
                    out=d["targ"][:], in0=d["tau"][r][:], scalar1=d["mu"][:, 0:1],
                    scalar2=INV_S2, op0=ALU.subtract, op1=ALU.mult)
            for bt in range(NBT):
                d = st[bt]
                nc.scalar.activation(
                    w[bt][:, 0:MA], z[bt][:, 0:MA], AF.Relu,
                    bias=d["ntau"][r][:, 0:1], accum_out=d["sact"][:])
                nc.scalar.activation(d["erf"][:], d["targ"][:], AF.Erf)
            for bt in range(NBT):
                d = st[bt]
                nc.vector.tensor_scalar(
                    out=d["kg"][:], in0=d["erf"][:], scalar1=-KHALF,
                    scalar2=KHALF, op0=ALU.mult, op1=ALU.add)
                nc.vector.tensor_scalar_max(d["kg"][:], d["kg"][:], 1.0)
                nc.vector.reciprocal(d["rk"][:], d["kg"][:])
            for bt in range(NBT):
                d = st[bt]
                tau_s = d["tau"][r][:, 0:1]
                # z - tau first (fp16 error vanishes near zero, where the
                # support lives), then relu with fused sum accumulation
                nc.vector.tensor_scalar(
                    out=scr[bt][:], in0=z[bt][:, MA:M],
                    scalar1=tau_s, scalar2=None, op0=ALU.subtract)
                nc.vector.tensor_scalar(
                    out=w[bt][:, MA:M], in0=scr[bt][:],
                    scalar1=0.0, scalar2=None,
                    op0=ALU.max, op1=ALU.add, accum_out=d["gacc"][:])
                nc.vector.tensor_add(d["sv"][:], d["sact"][:], d["gacc"][:])
                nc.vector.tensor_scalar(
                    out=d["step"][:], in0=d["sv"][:], scalar1=-1.0,
                    scalar2=d["rk"][:, 0:1], op0=ALU.add, op1=ALU.mult)
                nc.vector.tensor_scalar(
                    out=d["stepc"][:], in0=d["step"][:], scalar1=CLIP,
                    scalar2=-CLIP, op0=ALU.min, op1=ALU.max)
                nc.vector.tensor_scalar(
                    out=d["tau"][r + 1][:], in0=d["stepc"][:],
                    scalar1=d["tau"][r][:, 0:1], scalar2=d["cap"][:, 0:1],
                    op0=ALU.add, op1=ALU.min)
                nc.vector.tensor_scalar_mul(
                    d["ntau"][r + 1][:], d["tau"][r + 1][:], -1.0)
            # keep the PE clock from dropping to the lowest p-state
            for bt in range(NBT):
                wp = ps_warm.tile([P, P], F16, tag="warm")
                nc.tensor.transpose(wp[:], scr[bt][:, 0:P], ident[:])

        # ---- final w materialization at converged tau ----
        for bt in range(NBT):
            d = st[bt]
            nf = d["ntau"][N_ROUNDS]
            nc.scalar.activation(
                w[bt][:, 0:MA], z[bt][:, 0:MA], AF.Relu, bias=nf[:, 0:1])
        for bt in range(NBT):
            d = st[bt]
            tau_s = d["tau"][N_ROUNDS][:, 0:1]
            nc.vector.tensor_scalar(
                out=scr[bt][:], in0=z[bt][:, MA:M],
                scalar1=tau_s, scalar2=None, op0=ALU.subtract)
            nc.vector.tensor_scalar(
                out=w[bt][:, MA:M], in0=scr[bt][:],
                scalar1=0.0, scalar2=None, op0=ALU.max)
        ps_warm.release()
        mstream.release()
        zpool.release()

        # ---- w^T transposes + memory read: mv[bt] = w[bt] @ memR ----
        wTt = tc.alloc_tile_pool(name="wTt", bufs=12)
        mring = tc.alloc_tile_pool(name="mring", bufs=RING)
        w2s = tc.alloc_tile_pool(name="w2s", bufs=W2BUF)
        ps_tr = tc.alloc_tile_pool(name="ps_tr", bufs=4, space="PSUM")
        ps_mv = tc.alloc_tile_pool(name="ps_mv", bufs=1, space="PSUM")

        # prefill the ring (these transfers run during the sparsemax window)
        slabs = []
        for mc in range(MC):
            slab = mring.tile([P, D], F16, tag="memR", name=f"memR{mc}")
            slabs.append(slab)
            if mc < RING:
                nc.sync.dma_start(slab[:], memR[mc])
        # W2 on the gpsimd queue (ring-gated; nothing vital queued behind)
        w2t = [w2s.tile([P, OUT], F16, tag="w2", name=f"w2_{kc}")
               for kc in range(HC)]
        for kc in range(HC):
            nc.gpsimd.dma_start(w2t[kc][:], w2c[kc])

        mv_ps = [[ps_mv.tile([P, 512], F32, tag=f"mv{bt}_{dh}",
                             name=f"mv{bt}_{dh}")
                  for dh in range(2)] for bt in range(NBT)]
        # transposes run LAG chunks ahead of the read matmuls so the PE never
        # stalls on the cross-engine psum->sbuf evacuation roundtrip
        wTs = []
        for it in range(MC + LAG):
            if it < MC:
                mc = it
                if mc >= RING:
                    nc.sync.dma_start(slabs[mc][:], memR[mc])
                tp = ps_tr.tile([P, B_CORE], F16, tag="wtr")
                for bt in range(NBT):
                    nc.tensor.transpose(
                        tp[:, bt * P:(bt + 1) * P],
                        w[bt][:, mc * P:(mc + 1) * P], ident[:])
                wT = wTt.tile([P, B_CORE], F16, tag="wT", name=f"wT{mc}")
                wTs.append(wT)
                if mc % 2 == 0:
                    nc.vector.tensor_copy(wT[:], tp[:])
                else:
                    nc.scalar.copy(wT[:], tp[:])
            if it >= LAG:
                mc = it - LAG
                for bt in range(NBT):
                    for dh in range(2):
                        nc.tensor.matmul(
                            mv_ps[bt][dh][:], wTs[mc][:, bt * P:(bt + 1) * P],
                            slabs[mc][:, dh * 512:(dh + 1) * 512],
                            start=(mc == 0), stop=(mc == MC - 1))

        # ---- mv evac (fp16) + transpose to mvT [P, dc, 256] ----
        mv_sb = [small.tile([P, D], F16, tag=f"mvsb{bt}", name=f"mvsb{bt}")
                 for bt in range(NBT)]
        for bt in range(NBT):
            for dh in range(2):
                nc.scalar.copy(mv_sb[bt][:, dh * 512:(dh + 1) * 512],
                               mv_ps[bt][dh][:])
        ps_mv.release()
        mvT = small.tile([P, DC, B_CORE], F16, tag="mvT")
        for dc in range(DC):
            tp = ps_tr.tile([P, B_CORE], F16, tag="wtr")
            for bt in range(NBT):
                nc.tensor.transpose(
                    tp[:, bt * P:(bt + 1) * P],
                    mv_sb[bt][:, dc * P:(dc + 1) * P], ident[:])
            if dc % 2 == 0:
                nc.vector.tensor_copy(mvT[:, dc], tp[:])
            else:
                nc.scalar.copy(mvT[:, dc], tp[:])

        # ---- MLP1: h[bt] [P(b), H] = mvT-blocks^T @ W1-chunks (bias later) --
        hsb = [small.tile([P, H], F16, tag=f"h{bt}", name=f"h{bt}")
               for bt in range(NBT)]
        ps_h = tc.alloc_tile_pool(name="ps_h", bufs=1, space="PSUM")
        hps = [ps_h.tile([P, 512], F32, tag=f"hp{hb}", name=f"hp{hb}")
               for hb in range(HB)]
        for bt in range(NBT):
            for dc in range(DC):
                for hb in range(HB):
                    nc.tensor.matmul(
                        hps[hb][:], mvT[:, dc, bt * P:(bt + 1) * P],
                        w1t[dc][:, hb * 512:(hb + 1) * 512],
                        start=(dc == 0), stop=(dc == DC - 1))
            for hb in range(HB):
                nc.scalar.copy(hsb[bt][:, hb * 512:(hb + 1) * 512],
                               hps[hb][:])
        ps_h.release()

        # ---- hT transposes; relu + b1 fused into the per-partition evac ----
        hT = small.tile([P, HC, B_CORE], F16, tag="hT")
        for hc in range(HC):
            tp = ps_tr.tile([P, B_CORE], F16, tag="wtr")
            for bt in range(NBT):
                nc.tensor.transpose(
                    tp[:, bt * P:(bt + 1) * P],
                    hsb[bt][:, hc * P:(hc + 1) * P], ident[:])
            nc.scalar.activation(
                hT[:, hc], tp[:], AF.Relu, bias=b1t[:, hc:hc + 1])

        # ---- MLP2: out[bt] = hT-blocks^T @ W2 + b2 ----
        ps_o = tc.alloc_tile_pool(name="ps_o", bufs=1, space="PSUM")
        osb = [small.tile([P, OUT], F32, tag=f"osb{bt}", name=f"osb{bt}")
               for bt in range(NBT)]
        ops = [[ps_o.tile([P, NW], F32, tag=f"op{bt}_{oh}",
                          name=f"op{bt}_{oh}")
                for oh in range(NOH)] for bt in range(NBT)]
        for kc in range(HC):
            for bt in range(NBT):
                for oh in range(NOH):
                    nc.tensor.matmul(
                        ops[bt][oh][:], hT[:, kc, bt * P:(bt + 1) * P],
                        w2t[kc][:, oh * NW:(oh + 1) * NW],
                        start=(kc == 0), stop=False)
        for bt in range(NBT):
            for oh in range(NOH):
                nc.tensor.matmul(
                    ops[bt][oh][:], ones1[:], b2t[:, oh * NW:(oh + 1) * NW],
                    start=False, stop=True)
                nc.scalar.copy(osb[bt][:, oh * NW:(oh + 1) * NW],
                               ops[bt][oh][:])
            nc.sync.dma_start(out_d[bt], osb[bt][:])
        ps_o.release()
        ps_tr.release()
        w2s.release()
        mring.release()
        wTt.release()
        w1p.release()
        wpool.release()
        small.release()

    nc.compile()
    return nc


_CACHED = None


def _prep(inputs):
    x = np.ascontiguousarray(inputs["encoder_output"], dtype=np.float32)
    mem = np.ascontiguousarray(inputs["memory_set"], dtype=np.float32)
    W1 = np.ascontiguousarray(inputs["W1"], dtype=np.float32)
    b1 = np.ascontiguousarray(inputs["b1"], dtype=np.float32)
    W2 = np.ascontiguousarray(inputs["W2"], dtype=np.float32)
    b2 = np.ascontiguousarray(inputs["b2"], dtype=np.float32)

    inv_nx = 1.0 / np.sqrt((x * x).sum(1) + _EPS)
    inv_ny = 1.0 / np.sqrt((mem * mem).sum(1) + _EPS)
    xh = (x * inv_nx[:, None]).astype(np.float16)
    memT_hat = np.ascontiguousarray(
        (mem.T * inv_ny[None, :]).astype(np.float16).reshape(DC, P, M))
    memR_v = np.ascontiguousarray(mem.astype(np.float16).reshape(MC, P, D))
    w1_blk = np.ascontiguousarray(W1.astype(np.float16).reshape(DC, P, H))
    w2_blk = np.ascontiguousarray(W2.astype(np.float16).reshape(HC, P, OUT))
    b1_tiles = np.ascontiguousarray(b1.reshape(HC, P).T.astype(np.float32))
    b2_row = np.ascontiguousarray(b2.reshape(1, OUT).astype(np.float16))

    shared = {
        "memT": memT_hat, "memR": memR_v, "w1c": w1_blk,
        "w2c": w2_blk, "b1_t": b1_tiles, "b2_r": b2_row,
    }
    in_maps = []
    for c in range(8):
        xc = xh[c * B_CORE:(c + 1) * B_CORE]          # [256, 1024]
        xhT = np.ascontiguousarray(
            xc.T.reshape(DC, P, B_CORE).transpose(1, 0, 2)
            .reshape(P, DC * B_CORE))
        in_maps.append({"xhT": xhT, **shared})
    return in_maps


def kernel(**inputs) -> np.ndarray:
    global _CACHED
    if _CACHED is None:
        _CACHED = build()
    nc = _CACHED
    in_maps = _prep(inputs)
    res = run_bass_kernel_spmd(nc, in_maps, core_ids=list(range(8)))
    return np.concatenate(
        [r["out"].reshape(B_CORE, OUT) for r in res.results], axis=0)


# revision 15
# speedup vs baseline: 1.4168x; 1.0332x over previous
"""Trainium2 Bass kernel for nn_BaselineMemory (sparse attention memory read + MLP).

Data-parallel over batch: each of 8 NeuronCores handles 256 of 2048 rows.
fp16 end-to-end (fp32 PSUM/accum). Host pre-normalizes x and mem^T.
Pipeline per core:
  dist matmul z = x_hat @ y_hat^T (fp16, PE) -> z fp16 + block sums/maxes
  -> sparsemax tau via 3 Newton rounds from a Gaussian-moment init
     (S(tau) = ACT relu head + DVE tail; support count k on DVE is_ge;
      tau += (S-1)/k) -> w materialization pass
  -> w^T transposes (PE, fp16) -> memory read mv^T (fp16, PE)
  -> MLP1 (W1 natural layout, relu+b1 fused into post-transpose evac)
  -> MLP2 (+b2 via rank-1 matmul) -> fp32 out.
DMA: memT stream + memR ring + outputs on the Sync queue; W1/W2 on the
GpSimd queue so they transfer during the sparsemax window.
"""
import sys

if "/opt/trn_rl_repo" not in sys.path:
    sys.path.insert(0, "/opt/trn_rl_repo")

import numpy as np

import concourse.bass as bass  # noqa: F401
import concourse.tile as tile
from concourse import bacc, mybir
from concourse.bass_utils import run_bass_kernel_spmd
from concourse.masks import make_identity

P = 128
B_CORE = 256          # batch rows per core
NBT = B_CORE // P     # 2 b-tiles
D = 1024
DC = D // P           # 8 d-chunks
M = 8192
MB = M // 512         # 16 dist m-blocks per bt
MC = M // P           # 64 m-chunks for read
H = 2048
HC = H // P           # 16 h-chunks
HB = H // 512         # 4 mlp1 col-blocks
OUT = 1000
NOH = 2               # out halves of 500
NW = OUT // NOH

N_ROUNDS = 3          # quasi-Newton iterations on tau
MA = 5120             # S-pass head handled by ACT; tail [MA, M) on DVE
TAIL = M - MA
T0_SIG = 2.25 / 32.0  # init: tau0 = mean + 2.25*sigma, sigma = 1/sqrt(d)
INV_S2 = 22.627417    # 1/(sigma*sqrt(2)) = 32/sqrt(2)
KHALF = 4096.0        # m/2 for the erfc slope model
CLIP = 1.0 / 16.0     # step clip (2*sigma)
CAP_OFF = 1e-4
RING = 24             # memR ring slabs resident
W2BUF = 8
LAG = 8               # read matmuls trail w^T transposes by LAG chunks

F32 = mybir.dt.float32
F16 = mybir.dt.float16
AF = mybir.ActivationFunctionType
ALU = mybir.AluOpType
AX = mybir.AxisListType

_EPS = 1e-6


def build():
    nc = bacc.Bacc("TRN2", target_bir_lowering=False, debug=False)

    xhT_d = nc.dram_tensor("xhT", [P, DC * B_CORE], F16, kind="ExternalInput")
    memT = nc.dram_tensor("memT", [DC, P, M], F16, kind="ExternalInput")
    memR = nc.dram_tensor("memR", [MC, P, D], F16, kind="ExternalInput")
    w1c = nc.dram_tensor("w1c", [DC, P, H], F16, kind="ExternalInput")
    w2c = nc.dram_tensor("w2c", [HC, P, OUT], F16, kind="ExternalInput")
    b1_t = nc.dram_tensor("b1_t", [P, HC], F32, kind="ExternalInput")
    b2_r = nc.dram_tensor("b2_r", [1, OUT], F16, kind="ExternalInput")
    out_d = nc.dram_tensor("out", [NBT, P, OUT], F32, kind="ExternalOutput")

    with tile.TileContext(nc) as tc:
        small = tc.alloc_tile_pool(name="small", bufs=1)
        wpool = tc.alloc_tile_pool(name="wpool", bufs=1)
        w1p = tc.alloc_tile_pool(name="w1p", bufs=1)

        ident = small.tile([P, P], F16, tag="ident")
        make_identity(nc, ident[:])
        ones1 = small.tile([1, P], F16, tag="ones1")
        nc.vector.memset(ones1[:], 1.0)
        b1t = small.tile([P, HC], F32, tag="b1")
        nc.gpsimd.dma_start(b1t[:], b1_t[:])
        b2t = small.tile([1, OUT], F16, tag="b2")
        nc.gpsimd.dma_start(b2t[:], b2_r[:])
        xh = small.tile([P, DC * B_CORE], F16, tag="xh")
        nc.sync.dma_start(xh[:], xhT_d[:])

        w = [wpool.tile([P, M], F16, tag=f"w{bt}", name=f"w{bt}")
             for bt in range(NBT)]
        w1t = [w1p.tile([P, H], F16, tag=f"w1_{dc}", name=f"w1_{dc}")
               for dc in range(DC)]

        st = {}
        for bt in range(NBT):
            d = {}
            d["mx"] = small.tile([P, MB], F32, tag=f"mx{bt}", name=f"mx{bt}")
            d["zsum"] = small.tile([P, MB], F32, tag=f"zs{bt}", name=f"zs{bt}")
            for nm in ["rm", "cap", "zsr", "mu", "sact", "gacc", "targ",
                       "erf", "kg", "rk", "sv", "step", "stepc"]:
                d[nm] = small.tile([P, 1], F32, tag=f"{nm}{bt}", name=f"{nm}{bt}")
            d["tau"] = [small.tile([P, 1], F32, tag=f"tau{bt}_{r}",
                                   name=f"tau{bt}_{r}")
                        for r in range(N_ROUNDS + 1)]
            d["ntau"] = [small.tile([P, 1], F32, tag=f"ntau{bt}_{r}",
                                    name=f"ntau{bt}_{r}")
                         for r in range(N_ROUNDS + 1)]
            st[bt] = d

        # ---- persistent row tiles (released after w materialization) ----
        zpool = tc.alloc_tile_pool(name="zpool", bufs=1)
        z = [zpool.tile([P, M], F16, tag=f"z{bt}", name=f"z{bt}")
             for bt in range(NBT)]
        scr = [zpool.tile([P, TAIL], F16, tag=f"scr{bt}", name=f"scr{bt}")
               for bt in range(NBT)]

        # ---- PE warmup burst: ramp the clock while the first DMAs land ----
        junk = small.tile([P, 512], F16, tag="junk")
        nc.vector.memset(junk[:], 1.0)

        # ---- dist: z[bt] [P, M] fp16 + block sums/maxes ----
        mstream = tc.alloc_tile_pool(name="mstream", bufs=2)
        ps_wu = tc.alloc_tile_pool(name="ps_wu", bufs=2, space="PSUM")
        for i in range(10):
            wup = ps_wu.tile([P, 512], F32, tag="wu")
            nc.tensor.matmul(wup[:], ident[:], junk[:], start=True, stop=True)
        ps_wu.release()
        ps_dist = tc.alloc_tile_pool(name="ps_dist", bufs=4, space="PSUM")
        for blk in range(MB // 2):
            mt = mstream.tile([P, DC, 1024], F16, tag="memT")
            for dq in range(2):
                nc.sync.dma_start(
                    mt[:, dq * 4:(dq + 1) * 4],
                    memT[dq * 4:(dq + 1) * 4, :, blk * 1024:(blk + 1) * 1024]
                    .rearrange("d p m -> p d m"))
            for mh in range(2):
                mb = blk * 2 + mh
                for bt in range(NBT):
                    zp = ps_dist.tile([P, 512], F32, tag="zp")
                    for dc in range(DC):
                        nc.tensor.matmul(
                            zp[:],
                            xh[:, dc * B_CORE + bt * P:
                               dc * B_CORE + (bt + 1) * P],
                            mt[:, dc, mh * 512:(mh + 1) * 512],
                            start=(dc == 0), stop=(dc == DC - 1))
                    nc.scalar.activation(
                        z[bt][:, mb * 512:(mb + 1) * 512], zp[:], AF.Copy,
                        accum_out=st[bt]["zsum"][:, mb:mb + 1])
                    nc.vector.reduce_max(
                        st[bt]["mx"][:, mb:mb + 1],
                        z[bt][:, mb * 512:(mb + 1) * 512], axis=AX.X)
        ps_dist.release()


        # ---- sparsemax init: tau0 = mean + T0_SIG, capped below rowmax ----
        for bt in range(NBT):
            d = st[bt]
            nc.vector.reduce_sum(d["zsr"][:], d["zsum"][:], axis=AX.X)
            nc.vector.reduce_max(d["rm"][:], d["mx"][:], axis=AX.X)
            nc.vector.tensor_scalar_add(d["cap"][:], d["rm"][:], -CAP_OFF)
            nc.vector.tensor_scalar_mul(d["mu"][:], d["zsr"][:], 1.0 / M)
            nc.vector.tensor_scalar_add(d["step"][:], d["mu"][:], T0_SIG)
            nc.vector.tensor_tensor(
                d["tau"][0][:], d["step"][:], d["cap"][:], ALU.min)
            nc.vector.tensor_scalar_mul(d["ntau"][0][:], d["tau"][0][:], -1.0)

        ps_warm = tc.alloc_tile_pool(name="ps_warm", bufs=2, space="PSUM")

        # ---- quasi-Newton rounds: S(tau) measured, slope from the
        # Gaussian model k = m/2*erfc((tau-mu)/(sigma*sqrt2)) ----
        for r in range(N_ROUNDS):
            for bt in range(NBT):
                # slope model: depends only on tau[r] -> off the critical path
                d = st[bt]
                nc.vector.tensor_scalar(
                    out=d["targ"][:], in0=d["tau"][r][:], scalar1=d["mu"][:, 0:1],
                    scalar2=INV_S2, op0=ALU.subtract, op1=ALU.mult)
            for bt in range(NBT):
                d = st[bt]
                nc.scalar.activation(
                    w[bt][:, 0:MA], z[bt][:, 0:MA], AF.Relu,
                    bias=d["ntau"][r][:, 0:1], accum_out=d["sact"][:])
                nc.scalar.activation(d["erf"][:], d["targ"][:], AF.Erf)
            for bt in range(NBT):
                d = st[bt]
                nc.vector.tensor_scalar(
                    out=d["kg"][:], in0=d["erf"][:], scalar1=-KHALF,
                    scalar2=KHALF, op0=ALU.mult, op1=ALU.add)
                nc.vector.tensor_scalar_max(d["kg"][:], d["kg"][:], 1.0)
                nc.vector.reciprocal(d["rk"][:], d["kg"][:])
            for bt in range(NBT):
                d = st[bt]
                tau_s = d["tau"][r][:, 0:1]
                # z - tau first (fp16 error vanishes near zero, where the
                # support lives), then relu with fused sum accumulation
                nc.vector.tensor_scalar(
                    out=scr[bt][:], in0=z[bt][:, MA:M],
                    scalar1=tau_s, scalar2=None, op0=ALU.subtract)
                nc.vector.tensor_scalar(
                    out=w[bt][:, MA:M], in0=scr[bt][:],
                    scalar1=0.0, scalar2=None,
                    op0=ALU.max, op1=ALU.add, accum_out=d["gacc"][:])
                nc.vector.tensor_add(d["sv"][:], d["sact"][:], d["gacc"][:])
                nc.vector.tensor_scalar(
                    out=d["step"][:], in0=d["sv"][:], scalar1=-1.0,
                    scalar2=d["rk"][:, 0:1], op0=ALU.add, op1=ALU.mult)
                nc.vector.tensor_scalar(
                    out=d["stepc"][:], in0=d["step"][:], scalar1=CLIP,
                    scalar2=-CLIP, op0=ALU.min, op1=ALU.max)
                nc.vector.tensor_scalar(
                    out=d["tau"][r + 1][:], in0=d["stepc"][:],
                    scalar1=d["tau"][r][:, 0:1], scalar2=d["cap"][:, 0:1],
                    op0=ALU.add, op1=ALU.min)
                nc.vector.tensor_scalar_mul(
                    d["ntau"][r + 1][:], d["tau"][r + 1][:], -1.0)
            # keep the PE clock from dropping to the lowest p-state
            for bt in range(NBT):
                wp = ps_warm.tile([P, P], F16, tag="warm")
                nc.tensor.transpose(wp[:], scr[bt][:, 0:P], ident[:])

        # ---- final w materialization at converged tau ----
        for bt in range(NBT):
            d = st[bt]
            nf = d["ntau"][N_ROUNDS]
            nc.scalar.activation(
                w[bt][:, 0:MA], z[bt][:, 0:MA], AF.Relu, bias=nf[:, 0:1])
        for bt in range(NBT):
            d = st[bt]
            tau_s = d["tau"][N_ROUNDS][:, 0:1]
            nc.vector.tensor_scalar(
                out=scr[bt][:], in0=z[bt][:, MA:M],
                scalar1=tau_s, scalar2=None, op0=ALU.subtract)
            nc.vector.tensor_scalar(
                out=w[bt][:, MA:M], in0=scr[bt][:],
                scalar1=0.0, scalar2=None, op0=ALU.max)
        ps_warm.release()
        mstream.release()
        zpool.release()

        # ---- w^T transposes + memory read: mv[bt] = w[bt] @ memR ----
        wTt = tc.alloc_tile_pool(name="wTt", bufs=12)
        mring = tc.alloc_tile_pool(name="mring", bufs=3)
        w2s = tc.alloc_tile_pool(name="w2s", bufs=W2BUF)
        ps_tr = tc.alloc_tile_pool(name="ps_tr", bufs=4, space="PSUM")
        ps_mv = tc.alloc_tile_pool(name="ps_mv", bufs=1, space="PSUM")

        # memR ring: 8-slab groups (big DMAs amortize the trigger cost).
        # Prefill transfers run during the sparsemax window.
        GRP = 8
        NGRP = MC // GRP
        PRE = 3
        mgrp = []
        for g in range(NGRP):
            gt = mring.tile([P, GRP, D], F16, tag="memR", name=f"memR{g}")
            mgrp.append(gt)
            if g < PRE:
                nc.sync.dma_start(
                    gt[:], memR[g * GRP:(g + 1) * GRP]
                    .rearrange("c p d -> p c d"))
        # weights after the prefill: they arrive during the sparsemax window
        for dc in range(DC):
            nc.sync.dma_start(w1t[dc][:], w1c[dc])
        w2t = [w2s.tile([P, 2, OUT], F16, tag="w2", name=f"w2p_{g}")
               for g in range(HC // 2)]
        for g in range(HC // 2):
            nc.sync.dma_start(
                w2t[g][:],
                w2c[g * 2:(g + 1) * 2].rearrange("c p o -> p c o"))

        mv_ps = [[ps_mv.tile([P, 512], F32, tag=f"mv{bt}_{dh}",
                             name=f"mv{bt}_{dh}")
                  for dh in range(2)] for bt in range(NBT)]
        # transposes run LAG chunks ahead of the read matmuls so the PE never
        # stalls on the cross-engine psum->sbuf evacuation roundtrip
        wTs = []
        for it in range(MC + LAG):
            if it < MC:
                mc = it
                if mc % GRP == 0 and mc >= PRE * GRP:
                    g = mc // GRP
                    nc.sync.dma_start(
                        mgrp[g][:], memR[g * GRP:(g + 1) * GRP]
                        .rearrange("c p d -> p c d"))
                tp = ps_tr.tile([P, B_CORE], F16, tag="wtr")
                for bt in range(NBT):
                    nc.tensor.transpose(
                        tp[:, bt * P:(bt + 1) * P],
                        w[bt][:, mc * P:(mc + 1) * P], ident[:])
                wT = wTt.tile([P, B_CORE], F16, tag="wT", name=f"wT{mc}")
                wTs.append(wT)
                if mc % 2 == 0:
                    nc.vector.tensor_copy(wT[:], tp[:])
                else:
                    nc.scalar.copy(wT[:], tp[:])
            if it >= LAG:
                mc = it - LAG
                gt = mgrp[mc // GRP]
                sl = mc % GRP
                for bt in range(NBT):
                    for dh in range(2):
                        nc.tensor.matmul(
                            mv_ps[bt][dh][:], wTs[mc][:, bt * P:(bt + 1) * P],
                            gt[:, sl, dh * 512:(dh + 1) * 512],
                            start=(mc == 0), stop=(mc == MC - 1))

        # ---- mv evac (fp16) + transpose to mvT [P, dc, 256] ----
        mv_sb = [small.tile([P, D], F16, tag=f"mvsb{bt}", name=f"mvsb{bt}")
                 for bt in range(NBT)]
        for bt in range(NBT):
            for dh in range(2):
                nc.scalar.copy(mv_sb[bt][:, dh * 512:(dh + 1) * 512],
                               mv_ps[bt][dh][:])
        ps_mv.release()
        mvT = small.tile([P, DC, B_CORE], F16, tag="mvT")
        for dc in range(DC):
            tp = ps_tr.tile([P, B_CORE], F16, tag="wtr")
            for bt in range(NBT):
                nc.tensor.transpose(
                    tp[:, bt * P:(bt + 1) * P],
                    mv_sb[bt][:, dc * P:(dc + 1) * P], ident[:])
            if dc % 2 == 0:
                nc.vector.tensor_copy(mvT[:, dc], tp[:])
            else:
                nc.scalar.copy(mvT[:, dc], tp[:])

        # ---- MLP1: h[bt] [P(b), H] = mvT-blocks^T @ W1-chunks (bias later) --
        hsb = [small.tile([P, H], F16, tag=f"h{bt}", name=f"h{bt}")
               for bt in range(NBT)]
        ps_h = tc.alloc_tile_pool(name="ps_h", bufs=1, space="PSUM")
        hps = [ps_h.tile([P, 512], F32, tag=f"hp{hb}", name=f"hp{hb}")
               for hb in range(HB)]
        for bt in range(NBT):
            for dc in range(DC):
                for hb in range(HB):
                    nc.tensor.matmul(
                        hps[hb][:], mvT[:, dc, bt * P:(bt + 1) * P],
                        w1t[dc][:, hb * 512:(hb + 1) * 512],
                        start=(dc == 0), stop=(dc == DC - 1))
            for hb in range(HB):
                nc.scalar.copy(hsb[bt][:, hb * 512:(hb + 1) * 512],
                               hps[hb][:])
        ps_h.release()

        # ---- hT transposes; relu + b1 fused into the per-partition evac ----
        hT = small.tile([P, HC, B_CORE], F16, tag="hT")
        for hc in range(HC):
            tp = ps_tr.tile([P, B_CORE], F16, tag="wtr")
            for bt in range(NBT):
                nc.tensor.transpose(
                    tp[:, bt * P:(bt + 1) * P],
                    hsb[bt][:, hc * P:(hc + 1) * P], ident[:])
            nc.scalar.activation(
                hT[:, hc], tp[:], AF.Relu, bias=b1t[:, hc:hc + 1])

        # ---- MLP2: out[bt] = hT-blocks^T @ W2 + b2 ----
        ps_o = tc.alloc_tile_pool(name="ps_o", bufs=1, space="PSUM")
        osb = [small.tile([P, OUT], F32, tag=f"osb{bt}", name=f"osb{bt}")
               for bt in range(NBT)]
        ops = [[ps_o.tile([P, NW], F32, tag=f"op{bt}_{oh}",
                          name=f"op{bt}_{oh}")
                for oh in range(NOH)] for bt in range(NBT)]
        for kc in range(HC):
            for bt in range(NBT):
                for oh in range(NOH):
                    nc.tensor.matmul(
                        ops[bt][oh][:], hT[:, kc, bt * P:(bt + 1) * P],
                        w2t[kc // 2][:, kc % 2, oh * NW:(oh + 1) * NW],
                        start=(kc == 0), stop=False)
        for bt in range(NBT):
            for oh in range(NOH):
                nc.tensor.matmul(
                    ops[bt][oh][:], ones1[:], b2t[:, oh * NW:(oh + 1) * NW],
                    start=False, stop=True)
                nc.scalar.copy(osb[bt][:, oh * NW:(oh + 1) * NW],
                               ops[bt][oh][:])
            nc.sync.dma_start(out_d[bt], osb[bt][:])
        ps_o.release()
        ps_tr.release()
        w2s.release()
        mring.release()
        wTt.release()
        w1p.release()
        wpool.release()
        small.release()

    nc.compile()
    return nc


_CACHED = None


def _prep(inputs):
    x = np.ascontiguousarray(inputs["encoder_output"], dtype=np.float32)
    mem = np.ascontiguousarray(inputs["memory_set"], dtype=np.float32)
    W1 = np.ascontiguousarray(inputs["W1"], dtype=np.float32)
    b1 = np.ascontiguousarray(inputs["b1"], dtype=np.float32)
    W2 = np.ascontiguousarray(inputs["W2"], dtype=np.float32)
    b2 = np.ascontiguousarray(inputs["b2"], dtype=np.float32)

    inv_nx = 1.0 / np.sqrt((x * x).sum(1) + _EPS)
    inv_ny = 1.0 / np.sqrt((mem * mem).sum(1) + _EPS)
    xh = (x * inv_nx[:, None]).astype(np.float16)
    memT_hat = np.ascontiguousarray(
        (mem.T * inv_ny[None, :]).astype(np.float16).reshape(DC, P, M))
    memR_v = np.ascontiguousarray(mem.astype(np.float16).reshape(MC, P, D))
    w1_blk = np.ascontiguousarray(W1.astype(np.float16).reshape(DC, P, H))
    w2_blk = np.ascontiguousarray(W2.astype(np.float16).reshape(HC, P, OUT))
    b1_tiles = np.ascontiguousarray(b1.reshape(HC, P).T.astype(np.float32))
    b2_row = np.ascontiguousarray(b2.reshape(1, OUT).astype(np.float16))

    shared = {
        "memT": memT_hat, "memR": memR_v, "w1c": w1_blk,
        "w2c": w2_blk, "b1_t": b1_tiles, "b2_r": b2_row,
    }
    in_maps = []
    for c in range(8):
        xc = xh[c * B_CORE:(c + 1) * B_CORE]          # [256, 1024]
        xhT = np.ascontiguousarray(
            xc.T.reshape(DC, P, B_CORE).transpose(1, 0, 2)
            .reshape(P, DC * B_CORE))
        in_maps.append({"xhT": xhT, **shared})
    return in_maps


def kernel(**inputs) -> np.ndarray:
    global _CACHED
    if _CACHED is None:
        _CACHED = build()
    nc = _CACHED
    in_maps = _prep(inputs)
    res = run_bass_kernel_spmd(nc, in_maps, core_ids=list(range(8)))
    return np.concatenate(
        [r["out"].reshape(B_CORE, OUT) for r in res.results], axis=0)
